# revision 1
# baseline (speedup 1.0000x reference)
"""Causal self-attention for trn2, 8 NeuronCores.

Problem: x[4,2048,1024] @ w_qkv[1024,3072] -> causal MHA (16 heads, d=64)
-> @ w_out[1024,1024].

Sharding: core c handles batch b=c%4 and heads hbase=8*(c//4)..hbase+8
(data parallel on B x tensor parallel on heads). Each core computes the
partial out-projection y_c = att_slice @ w_out[slice]; the host sums the
two partials per batch.

v4: all matmul operands bf16 (fp32 PSUM accumulation). x is cast to a
ct-major bf16 DRAM scratch (SWDGE cast-DMA, contiguous [2048,128] blocks)
and transposed with hardware DMA-transpose loads. All weights are cast
once into resident bf16 tiles by SWDGE cast-DMAs. Softmax denominators
come from a fused ones-column in the AV matmul ([V|1]^T w^T row 64);
causal masking skips above-diagonal tiles and applies one gpsimd
affine_select per diagonal 128x128 block after the exp. Normalization:
DVE reciprocal + DRAM-bounce partition broadcast + multiply, staged off
PSUM so nothing blocks the accumulators.

4-round pipeline over T-quarters: round r transposes quarter r, projects
qT/kT/V for it, runs attention q-block r for every head (causality needs
only k/V quarters <= r), then the out-projection for those q rows. PSUM:
sA/sB double-buffered [128,512] scores, av_A/av_B accumulators, and a
dedicated [128,1024] projection tag so next-round projection matmuls can
fill TensorE gaps while ScalarE paces the attention exps.
"""

import sys

for p in ("/opt/trn_rl_repo", "/opt/pypackages"):
    if p not in sys.path:
        sys.path.insert(0, p)

import contextlib

import numpy as np

import concourse.bass as bass
import concourse.mybir as mybir
import concourse.tile as tile
from concourse import bacc
from concourse.bass_utils import run_bass_kernel_spmd
from concourse.masks import make_identity

F32 = mybir.dt.float32
BF = mybir.dt.bfloat16
EXP = mybir.ActivationFunctionType.Exp

T = 2048          # sequence length
C = 1024          # model dim
HC = 8            # heads per core
D = 64            # head dim
NG = 4            # head-groups of 2 per core
NCT = C // 128    # 8 contraction tiles
NTT = T // 128    # 16 token tiles
SCALE = 0.125     # 1/sqrt(D)


def build_nc():
    nc = bacc.Bacc("TRN2", target_bir_lowering=False, debug=False)

    x_d = nc.dram_tensor("x", [T, C], F32, kind="ExternalInput")
    wq_d = nc.dram_tensor("wq", [C, 512], F32, kind="ExternalInput")
    wk_d = nc.dram_tensor("wk", [C, 512], F32, kind="ExternalInput")
    wv_d = nc.dram_tensor("wv", [C, 512], F32, kind="ExternalInput")
    wo_d = nc.dram_tensor("wo", [512, C], F32, kind="ExternalInput")
    y_d = nc.dram_tensor("y", [T, C], F32, kind="ExternalOutput")

    with tile.TileContext(nc) as tc, contextlib.ExitStack() as ctx:
        persist = ctx.enter_context(tc.tile_pool(name="persist", bufs=1))
        work = ctx.enter_context(tc.tile_pool(name="work", bufs=1))
        ps = ctx.enter_context(tc.tile_pool(name="ps", bufs=1, space="PSUM"))
        dpool = ctx.enter_context(tc.tile_pool(name="dram", bufs=1, space="DRAM"))

        kT = [persist.tile([128, T], BF, tag=f"kT{g}", name=f"kT{g}")
              for g in range(NG)]
        V = persist.tile([128, NTT, HC, 65], BF, tag="V")

        # x -> bf16 DRAM scratch. The cast must be a CONTIGUOUS SWDGE DMA:
        # strided cast-DMAs truncate instead of round-to-nearest, and the
        # truncation bias blows up the dot products downstream.
        xbf = dpool.tile([T, C], BF, tag="xbf", name="xbf")
        # round 0's xT comes from on-chip PE transposes so TensorE starts
        # within ~10us instead of waiting for the cast->DMA-transpose chain;
        # rounds 1-3 still use the cheap hardware DMA-transpose path.
        ident = persist.tile([128, 128], F32, tag="ident", name="ident")
        make_identity(nc, ident)
        xTq0 = [work.tile([128, 512], BF, tag=f"xTq{ct}", name=f"xTq{ct}",
                          bufs=2)
                for ct in range(NCT)]
        for j in range(4):
            x_nat = work.tile([128, C], F32, tag="x_nat", bufs=2, name="x_nat")
            nc.sync.dma_start(out=x_nat, in_=x_d.ap()[j * 128:(j + 1) * 128, :])
            tp0 = ps.tile([128, 1024], F32, tag="sc", bufs=2, name="tp0")
            for ct in range(NCT):
                nc.tensor.transpose(
                    tp0[:, ct * 128:(ct + 1) * 128],
                    x_nat[:, ct * 128:(ct + 1) * 128],
                    ident,
                )
            for ct in range(NCT):
                nc.vector.tensor_copy(
                    xTq0[ct][:, j * 128:(j + 1) * 128],
                    tp0[:, ct * 128:(ct + 1) * 128],
                )
        # qkv weights: direct f32 loads + DVE casts so round-0 projection
        # is never stuck behind the SWDGE cast chain; wo (needed latest)
        # keeps the DRAM-bounce cast.
        wq_bf = persist.tile([128, NCT, 512], BF, tag="wq_bf")
        wk_bf = persist.tile([128, NCT, 512], BF, tag="wk_bf")
        wv_bf = persist.tile([128, NCT, 512], BF, tag="wv_bf")
        for wdram, wbf in ((wq_d, wq_bf), (wk_d, wk_bf), (wv_d, wv_bf)):
            wstage = work.tile([128, NCT, 512], F32, tag="wstage", name="wstage")
            nc.sync.dma_start(
                out=wstage, in_=wdram.ap().rearrange("(ct p) m -> p ct m", p=128))
            nc.vector.tensor_copy(wbf, wstage)
        wod_bf = dpool.tile([512, C], BF, tag="wod_bf", name="wod_bf")
        nc.gpsimd.dma_start(out=wod_bf, in_=wo_d.ap())
        wo_bf = persist.tile([128, NG, C], BF, tag="wo_bf")
        nc.sync.dma_start(
            out=wo_bf, in_=wod_bf.rearrange("(g p) c -> p g c", p=128))

        for rnd in range(1, 4):
            nc.gpsimd.dma_start(
                out=xbf[rnd * 512:(rnd + 1) * 512, :],
                in_=x_d.ap()[rnd * 512:(rnd + 1) * 512, :],
            )
        # quarter 0 of xbf is unused now (round 0 transposed on-chip)

        # ones column of V
        ones_f32 = persist.tile([128, NTT, HC], F32, tag="ones")
        nc.vector.memset(ones_f32, 1.0)
        nc.vector.tensor_copy(V[:, :, :, 64], ones_f32)

        for rnd in range(4):
            q0 = rnd * 512  # first token of this quarter

            # ---- xT quarter via hardware DMA-transpose ----
            if rnd == 0:
                xTq = xTq0
            else:
                xTq = [work.tile([128, 512], BF, tag=f"xTq{ct}",
                                 name=f"xTq{ct}", bufs=2)
                       for ct in range(NCT)]
                for ct in range(NCT):
                    nc.sync.dma_start_transpose(
                        out=xTq[ct],
                        in_=xbf[q0:q0 + 512, ct * 128:(ct + 1) * 128]
                    )

            # ---- qT/kT for this quarter ----
            qTq = []
            for g in range(NG):
                pqk = ps.tile([128, 1024], F32, tag="pp", name="pqk")
                for ct in range(NCT):
                    nc.tensor.matmul(
                        pqk[:, 0:512],
                        wq_bf[:, ct, g * 128:(g + 1) * 128],
                        xTq[ct],
                        start=(ct == 0), stop=(ct == NCT - 1),
                    )
                    nc.tensor.matmul(
                        pqk[:, 512:1024],
                        wk_bf[:, ct, g * 128:(g + 1) * 128],
                        xTq[ct],
                        start=(ct == 0), stop=(ct == NCT - 1),
                    )
                qq = work.tile([128, 512], BF, tag=f"qTq{g}", bufs=2,
                               name=f"qTq{g}")
                nc.vector.tensor_copy(qq, pqk[:, 0:512])
                qTq.append(qq)
                nc.vector.tensor_copy(kT[g][:, q0:q0 + 512], pqk[:, 512:1024])

            # ---- V for this quarter (two tt-pairs per psum tile) ----
            for half in range(2):
                pv = ps.tile([128, 1024], F32, tag="pp", name="pv")
                for ct in range(NCT):
                    for sub in range(2):
                        jl = half * 2 + sub
                        nc.tensor.matmul(
                            pv[:, sub * 512:(sub + 1) * 512],
                            xTq[ct][:, jl * 128:(jl + 1) * 128],
                            wv_bf[:, ct, :],
                            start=(ct == 0), stop=(ct == NCT - 1),
                        )
                for sub in range(2):
                    tt = rnd * 4 + half * 2 + sub
                    for h in range(HC):
                        nc.vector.tensor_copy(
                            V[:, tt, h, 0:64],
                            pv[:, sub * 512 + h * 64: sub * 512 + h * 64 + 64],
                        )

            # ---- attention: q-block rnd for every group ----
            # Heads sequential, 2-kt score batches: 2-matmul bursts into a
            # [128,1024] psum span, one exp, causal select on diagonal
            # blocks, then a 2-matmul AV burst.
            qb = rnd
            nkt = 4 * (qb + 1)
            attTq = []
            for g in range(NG):
                att = work.tile([128, 512], BF, tag=f"attTq{g}", bufs=2,
                                name=f"attTq{g}")
                for hh in range(2):
                    head = 2 * g + hh
                    r0, r1 = 64 * hh, 64 * hh + 64
                    tp = (64 * hh, 0)
                    av = ps.tile([65, 512], F32, tag=f"av{hh}", name="av")
                    for b0 in range(0, nkt, 2):
                        sc = ps.tile([128, 1024], F32, tag="sc", bufs=2, name="sc")
                        for m in range(2):
                            nc.tensor.matmul(
                                sc[:, m * 512:(m + 1) * 512],
                                kT[g][r0:r1, (b0 + m) * 128:(b0 + m + 1) * 128],
                                qTq[g][r0:r1, :],
                                start=True, stop=True,
                                tile_position=tp,
                            )
                        wT = work.tile([128, 1024], BF, tag="wT", bufs=3)
                        nc.scalar.activation(wT, sc, EXP, scale=SCALE)
                        for m in range(2):
                            j = b0 + m - 4 * qb
                            if j >= 0:  # diagonal 128-block: causal select
                                ncols = 128 * j + 128
                                nc.gpsimd.affine_select(
                                    out=wT[:, m * 512:m * 512 + ncols],
                                    in_=wT[:, m * 512:m * 512 + ncols],
                                    compare_op=mybir.AluOpType.is_ge,
                                    fill=0.0,
                                    base=-128 * j,
                                    pattern=[[1, ncols]],
                                    channel_multiplier=-1,
                                )
                        for m in range(2):
                            kt = b0 + m
                            nc.tensor.matmul(
                                av, V[:, kt, head, :],
                                wT[:, m * 512:(m + 1) * 512],
                                start=(kt == 0), stop=(kt == nkt - 1),
                            )
                    # stage off PSUM, normalize off the critical path
                    avc = work.tile([65, 512], F32, tag="avc", bufs=4, name="avc")
                    nc.vector.tensor_copy(avc, av)
                    rec = work.tile([65, 512], F32, tag="rec", bufs=4, name="rec")
                    nc.vector.reciprocal(rec[64:65, :], avc[64:65, :])
                    rec_d = dpool.tile([1, 512], F32, tag="rec_d", bufs=4,
                                       name="rec_d")
                    nc.sync.dma_start(out=rec_d, in_=rec[64:65, :])
                    rep = work.tile([64, 512], F32, tag="rep", bufs=4, name="rep")
                    nc.sync.dma_start(
                        out=rep,
                        in_=bass.AP(rec_d.tensor, rec_d.offset,
                                    [[0, 64], [1, 512]]),
                    )
                    if hh == 0:
                        nc.vector.tensor_mul(att[0:64, :], avc[0:64, :], rep)
                    else:
                        tmpB = work.tile([64, 512], BF, tag="tmpB", bufs=2,
                                         name="tmpB")
                        nc.vector.tensor_mul(tmpB, avc[0:64, :], rep)
                        nc.sync.dma_start(out=att[64:128, :], in_=tmpB)
                attTq.append(att)

            # ---- out projection for this quarter's q rows ----
            for qtl in range(4):
                qt = rnd * 4 + qtl
                psy = ps.tile([128, 1024], F32, tag="pp", name="psy")
                for g in range(NG):
                    for half in range(2):
                        nc.tensor.matmul(
                            psy[:, half * 512:(half + 1) * 512],
                            attTq[g][:, qtl * 128:(qtl + 1) * 128],
                            wo_bf[:, g, half * 512:(half + 1) * 512],
                            start=(g == 0),
                            stop=(g == NG - 1),
                        )
                y_sb = work.tile([128, C], F32, tag="y_sb", bufs=2, name="y_sb")
                nc.vector.tensor_copy(y_sb, psy)
                nc.sync.dma_start(
                    out=y_d.ap()[qt * 128:(qt + 1) * 128, :], in_=y_sb
                )

    nc.compile()
    return nc


_NC_CACHE = None


def _get_nc():
    global _NC_CACHE
    if _NC_CACHE is None:
        _NC_CACHE = build_nc()
    return _NC_CACHE


def kernel(x, w_qkv, w_out, _trace=False):
    B = x.shape[0]
    x = np.ascontiguousarray(x, dtype=np.float32)
    w_qkv = np.ascontiguousarray(w_qkv, dtype=np.float32)
    w_out = np.ascontiguousarray(w_out, dtype=np.float32)

    nc = _get_nc()
    in_maps = []
    for core in range(8):
        b = core % B
        hbase = (core // B) * HC
        lo, hi = hbase * D, hbase * D + HC * D
        in_maps.append({
            "x": x[b],
            "wq": np.ascontiguousarray(w_qkv[:, lo:hi]),
            "wk": np.ascontiguousarray(w_qkv[:, C + lo:C + hi]),
            "wv": np.ascontiguousarray(w_qkv[:, 2 * C + lo:2 * C + hi]),
            "wo": np.ascontiguousarray(w_out[lo:hi, :]),
        })

    res = run_bass_kernel_spmd(nc, in_maps, core_ids=list(range(8)), trace=_trace)
    ys = [r["y"] for r in res.results]
    out = np.empty((B, T, C), dtype=np.float32)
    for b in range(B):
        out[b] = ys[b] + ys[b + B]
    if _trace:
        return out, res
    return out



# revision 7
# speedup vs baseline: 1.2507x; 1.2507x over previous
"""Causal self-attention for trn2, 8 NeuronCores.

Problem: x[4,2048,1024] @ w_qkv[1024,3072] -> causal MHA (16 heads, d=64)
-> @ w_out[1024,1024].

Sharding: core c handles batch b=c%4 and heads hbase=8*(c//4)..hbase+8
(data parallel on B x tensor parallel on heads). Each core computes the
partial out-projection y_c = att_slice @ w_out[slice]; the host sums the
two partials per batch.

v5: attention processes both heads of a group together per kt-pair unit
so the K=64 score matmuls pack into concurrent PE row-bands
(tile_position (0,0)/(64,0)) and exp(h0) overlaps matmuls(h1). Diagonal
kt tiles compute only causally-valid query columns in a packed layout
(saves ~15% ScalarE exp work + PE cycles) with a uniform [128,128]
triangle affine_select. Softmax denominators: ones-row in the AV matmul,
DRAM-bounce partition broadcast, then reciprocal_approx_fast on the
broadcast [64,512] tile (5x cheaper than DVE reciprocal). PSUM tags:
sc0/sc1 (2 banks each, per-head score tiles), av0/av1 (1 each), proj
(2, qkv projections); out-projection reuses the sc tags at round end so
next-round projections can overlap attention.
"""

import sys

for p in ("/opt/trn_rl_repo", "/opt/pypackages"):
    if p not in sys.path:
        sys.path.insert(0, p)

import contextlib

import numpy as np

import concourse.bass as bass
import concourse.mybir as mybir
import concourse.tile as tile
from concourse import bacc
from concourse.bass_utils import run_bass_kernel_spmd
from concourse.masks import make_identity

F32 = mybir.dt.float32
BF = mybir.dt.bfloat16
EXP = mybir.ActivationFunctionType.Exp

T = 2048          # sequence length
C = 1024          # model dim
HC = 8            # heads per core
D = 64            # head dim
NG = 4            # head-groups of 2 per core
NCT = C // 128    # 8 contraction tiles
NTT = T // 128    # 16 token tiles
SCALE = 0.125     # 1/sqrt(D)


def build_nc():
    nc = bacc.Bacc("TRN2", target_bir_lowering=False, debug=False)

    x_d = nc.dram_tensor("x", [T, C], F32, kind="ExternalInput")
    wq_d = nc.dram_tensor("wq", [C, 512], F32, kind="ExternalInput")
    wk_d = nc.dram_tensor("wk", [C, 512], F32, kind="ExternalInput")
    wv_d = nc.dram_tensor("wv", [C, 512], F32, kind="ExternalInput")
    wo_d = nc.dram_tensor("wo", [512, C], F32, kind="ExternalInput")
    y_d = nc.dram_tensor("y", [T, C], F32, kind="ExternalOutput")

    with tile.TileContext(nc) as tc, contextlib.ExitStack() as ctx:
        persist = ctx.enter_context(tc.tile_pool(name="persist", bufs=1))
        work = ctx.enter_context(tc.tile_pool(name="work", bufs=1))
        ps = ctx.enter_context(tc.tile_pool(name="ps", bufs=1, space="PSUM"))
        dpool = ctx.enter_context(tc.tile_pool(name="dram", bufs=1, space="DRAM"))

        kT = [persist.tile([128, T], BF, tag=f"kT{g}", name=f"kT{g}")
              for g in range(NG)]
        V = persist.tile([128, NTT, HC, 65], BF, tag="V")

        # x -> bf16 DRAM scratch. The cast must be a CONTIGUOUS SWDGE DMA:
        # strided cast-DMAs truncate instead of round-to-nearest, and the
        # truncation bias blows up the dot products downstream.
        xbf = dpool.tile([T, C], BF, tag="xbf", name="xbf")
        # round 0's xT comes from on-chip PE transposes so TensorE starts
        # within ~10us instead of waiting for the cast->DMA-transpose chain;
        # rounds 1-3 still use the cheap hardware DMA-transpose path.
        ident = persist.tile([128, 128], F32, tag="ident", name="ident")
        make_identity(nc, ident)
        xTq0 = [work.tile([128, 512], BF, tag=f"xTq{ct}", name=f"xTq{ct}",
                          bufs=2)
                for ct in range(NCT)]
        for j in range(4):
            x_nat = work.tile([128, C], F32, tag="x_nat", bufs=2, name="x_nat")
            nc.sync.dma_start(out=x_nat, in_=x_d.ap()[j * 128:(j + 1) * 128, :])
            tp0 = ps.tile([128, 1024], F32, tag=f"sc{j % 2}", name="tp0")
            for ct in range(NCT):
                nc.tensor.transpose(
                    tp0[:, ct * 128:(ct + 1) * 128],
                    x_nat[:, ct * 128:(ct + 1) * 128],
                    ident,
                )
            for ct in range(NCT):
                nc.vector.tensor_copy(
                    xTq0[ct][:, j * 128:(j + 1) * 128],
                    tp0[:, ct * 128:(ct + 1) * 128],
                )
        # qkv weights: direct f32 loads + DVE casts so round-0 projection
        # is never stuck behind the SWDGE cast chain; wo (needed latest)
        # keeps the DRAM-bounce cast.
        wq_bf = persist.tile([128, NCT, 512], BF, tag="wq_bf")
        wk_bf = persist.tile([128, NCT, 512], BF, tag="wk_bf")
        wv_bf = persist.tile([128, NCT, 512], BF, tag="wv_bf")
        for wdram, wbf in ((wq_d, wq_bf), (wk_d, wk_bf), (wv_d, wv_bf)):
            wstage = work.tile([128, NCT, 512], F32, tag="wstage", name="wstage")
            nc.sync.dma_start(
                out=wstage, in_=wdram.ap().rearrange("(ct p) m -> p ct m", p=128))
            nc.vector.tensor_copy(wbf, wstage)
        wod_bf = dpool.tile([512, C], BF, tag="wod_bf", name="wod_bf")
        nc.gpsimd.dma_start(out=wod_bf, in_=wo_d.ap())
        wo_bf = persist.tile([128, NG, C], BF, tag="wo_bf")
        nc.sync.dma_start(
            out=wo_bf, in_=wod_bf.rearrange("(g p) c -> p g c", p=128))

        for rnd in range(1, 4):
            nc.gpsimd.dma_start(
                out=xbf[rnd * 512:(rnd + 1) * 512, :],
                in_=x_d.ap()[rnd * 512:(rnd + 1) * 512, :],
            )
        # quarter 0 of xbf is unused now (round 0 transposed on-chip)

        # ones column of V
        ones_f32 = persist.tile([128, NTT, HC], F32, tag="ones")
        nc.vector.memset(ones_f32, 1.0)
        nc.vector.tensor_copy(V[:, :, :, 64], ones_f32)

        for rnd in range(4):
            q0 = rnd * 512  # first token of this quarter
            nkt = 4 * (rnd + 1)

            # ---- xT quarter via hardware DMA-transpose ----
            if rnd == 0:
                xTq = xTq0
            else:
                xTq = [work.tile([128, 512], BF, tag=f"xTq{ct}",
                                 name=f"xTq{ct}", bufs=2)
                       for ct in range(NCT)]
                for ct in range(NCT):
                    nc.sync.dma_start_transpose(
                        out=xTq[ct],
                        in_=xbf[q0:q0 + 512, ct * 128:(ct + 1) * 128]
                    )

            # ---- qT/kT for this quarter ----
            qTq = []
            for g in range(NG):
                pqk = ps.tile([128, 1024], F32, tag="proj", name="pqk")
                for ct in range(NCT):
                    nc.tensor.matmul(
                        pqk[:, 0:512],
                        wq_bf[:, ct, g * 128:(g + 1) * 128],
                        xTq[ct],
                        start=(ct == 0), stop=(ct == NCT - 1),
                    )
                    nc.tensor.matmul(
                        pqk[:, 512:1024],
                        wk_bf[:, ct, g * 128:(g + 1) * 128],
                        xTq[ct],
                        start=(ct == 0), stop=(ct == NCT - 1),
                    )
                qq = work.tile([128, 512], BF, tag=f"qTq{g}", bufs=2,
                               name=f"qTq{g}")
                nc.vector.tensor_copy(qq, pqk[:, 0:512])
                qTq.append(qq)
                nc.vector.tensor_copy(kT[g][:, q0:q0 + 512], pqk[:, 512:1024])

            # ---- V for this quarter (two tt-pairs per psum tile) ----
            for half in range(2):
                pv = ps.tile([128, 1024], F32, tag="proj", name="pv")
                for ct in range(NCT):
                    for sub in range(2):
                        jl = half * 2 + sub
                        nc.tensor.matmul(
                            pv[:, sub * 512:(sub + 1) * 512],
                            xTq[ct][:, jl * 128:(jl + 1) * 128],
                            wv_bf[:, ct, :],
                            start=(ct == 0), stop=(ct == NCT - 1),
                        )
                tt0 = rnd * 4 + half * 2
                nc.vector.tensor_copy(
                    V[:, tt0:tt0 + 2, :, 0:64],
                    pv.rearrange("p (t h d) -> p t h d", t=2, h=HC),
                )

            # ---- attention: q-block rnd for every group ----
            # Both heads of a group advance together through kt-pair units:
            # the two K=64 score matmuls of a pair pack into PE row-bands
            # 0-63 / 64-127 and run concurrently; exp(h0) overlaps AV(h1).
            # Diagonal kt tiles compute only valid query columns, written
            # at packed offsets so one exp instruction covers them.
            avcs = {}
            for g in range(NG):
                av = [ps.tile([65, 512], F32, tag=f"av{hh}", name=f"av{hh}")
                      for hh in range(2)]
                for b0 in range(0, nkt, 2):
                    jb = b0 - 4 * rnd  # >=0 on the two diagonal units
                    # (m, q-start, wT col offset, ncols, triangle col or None,
                    #  start, stop) per kt of the unit
                    if jb < 0:
                        plan = [(0, 0, 0, 512, None, True, True),
                                (1, 0, 512, 512, None, True, True)]
                        expw = 1024
                    elif jb == 0:
                        plan = [(0, 0, 0, 512, 0, True, True),
                                (1, 128, 512, 384, 512, True, True)]
                        expw = 896
                    else:  # jb == 2: both kts land in psum bank 0 -> one
                        # accumulation group writing disjoint column ranges
                        plan = [(0, 256, 0, 256, 0, True, False),
                                (1, 384, 256, 128, 256, False, True)]
                        expw = 384
                    sc = [ps.tile([128, 1024], F32, tag=f"sc{hh}",
                                  name=f"sc{hh}") for hh in range(2)]
                    for (m, qs, co, ncol, tri, st, sp) in plan:
                        for hh in range(2):
                            r0 = 64 * hh
                            nc.tensor.matmul(
                                sc[hh][:, co:co + ncol],
                                kT[g][r0:r0 + 64,
                                      (b0 + m) * 128:(b0 + m + 1) * 128],
                                qTq[g][r0:r0 + 64, qs:qs + ncol],
                                start=st, stop=sp,
                                tile_position=(r0, 0),
                            )
                    wT = [work.tile([128, 1024], BF, tag=f"wT{hh}", bufs=2,
                                    name=f"wT{hh}") for hh in range(2)]
                    for hh in range(2):
                        nc.scalar.activation(wT[hh][:, 0:expw],
                                             sc[hh][:, 0:expw],
                                             EXP, scale=SCALE)
                    for (m, qs, co, ncol, tri, st, sp) in plan:
                        if tri is None:
                            continue
                        for hh in range(2):
                            nc.gpsimd.affine_select(
                                out=wT[hh][:, tri:tri + 128],
                                in_=wT[hh][:, tri:tri + 128],
                                compare_op=mybir.AluOpType.is_ge,
                                fill=0.0,
                                base=0,
                                pattern=[[1, 128]],
                                channel_multiplier=-1,
                            )
                    for hh in range(2):
                        head = 2 * g + hh
                        for (m, qs, co, ncol, tri, st, sp) in plan:
                            kt = b0 + m
                            nc.tensor.matmul(
                                av[hh][:, qs:qs + ncol],
                                V[:, kt, head, :],
                                wT[hh][:, co:co + ncol],
                                start=(kt == 0), stop=(kt == nkt - 1),
                            )
                # stage AV (with its ones-row denominator) off PSUM;
                # DRAM-bounce broadcasts the raw denominator to 64
                # partitions, then one fast approximate reciprocal at
                # partition base 0 replaces the 3.3us DVE reciprocal
                for hh in range(2):
                    avc = work.tile([65, 512], F32, tag="avc", bufs=8,
                                    name="avc")
                    nc.vector.tensor_copy(avc, av[hh])
                    avcs[(g, hh)] = avc

            attTq = []
            for g in range(NG):
                att = work.tile([128, 512], BF, tag=f"attTq{g}", bufs=2,
                                name=f"attTq{g}")
                for hh in range(2):
                    avc = avcs[(g, hh)]
                    den_d = dpool.tile([1, 512], F32, tag="den_d", bufs=8,
                                       name="den_d")
                    nc.sync.dma_start(out=den_d, in_=avc[64:65, :])
                    den_b = work.tile([64, 512], F32, tag="den_b", bufs=4,
                                      name="den_b")
                    nc.sync.dma_start(
                        out=den_b,
                        in_=bass.AP(den_d.tensor, den_d.offset,
                                    [[0, 64], [1, 512]]),
                    )
                    rep = work.tile([64, 512], F32, tag="rep", bufs=4,
                                    name="rep")
                    nc.vector.reciprocal_approx_fast(out=rep, in_=den_b)
                    if hh == 0:
                        nc.vector.tensor_mul(att[0:64, :], avc[0:64, :], rep)
                    else:
                        tmpB = work.tile([64, 512], BF, tag="tmpB", bufs=2,
                                         name="tmpB")
                        nc.vector.tensor_mul(tmpB, avc[0:64, :], rep)
                        nc.sync.dma_start(out=att[64:128, :], in_=tmpB)
                attTq.append(att)

            # ---- out projection for this quarter's q rows ----
            # psum comes from the sc tags (attention is done with them)
            for qtl in range(4):
                qt = rnd * 4 + qtl
                psy = ps.tile([128, 1024], F32, tag=f"sc{qtl % 2}",
                              name="psy")
                for g in range(NG):
                    for half in range(2):
                        nc.tensor.matmul(
                            psy[:, half * 512:(half + 1) * 512],
                            attTq[g][:, qtl * 128:(qtl + 1) * 128],
                            wo_bf[:, g, half * 512:(half + 1) * 512],
                            start=(g == 0),
                            stop=(g == NG - 1),
                        )
                y_sb = work.tile([128, C], F32, tag="y_sb", bufs=2, name="y_sb")
                nc.vector.tensor_copy(y_sb, psy)
                nc.sync.dma_start(
                    out=y_d.ap()[qt * 128:(qt + 1) * 128, :], in_=y_sb
                )

    nc.compile()
    return nc


_NC_CACHE = None


def _get_nc():
    global _NC_CACHE
    if _NC_CACHE is None:
        _NC_CACHE = build_nc()
    return _NC_CACHE


def kernel(x, w_qkv, w_out, _trace=False):
    B = x.shape[0]
    x = np.ascontiguousarray(x, dtype=np.float32)
    w_qkv = np.ascontiguousarray(w_qkv, dtype=np.float32)
    w_out = np.ascontiguousarray(w_out, dtype=np.float32)

    nc = _get_nc()
    in_maps = []
    for core in range(8):
        b = core % B
        hbase = (core // B) * HC
        lo, hi = hbase * D, hbase * D + HC * D
        in_maps.append({
            "x": x[b],
            "wq": np.ascontiguousarray(w_qkv[:, lo:hi]),
            "wk": np.ascontiguousarray(w_qkv[:, C + lo:C + hi]),
            "wv": np.ascontiguousarray(w_qkv[:, 2 * C + lo:2 * C + hi]),
            "wo": np.ascontiguousarray(w_out[lo:hi, :]),
        })

    res = run_bass_kernel_spmd(nc, in_maps, core_ids=list(range(8)), trace=_trace)
    ys = [r["y"] for r in res.results]
    out = np.empty((B, T, C), dtype=np.float32)
    for b in range(B):
        out[b] = ys[b] + ys[b + B]
    if _trace:
        return out, res
    return out


# revision 13
# speedup vs baseline: 1.3253x; 1.0596x over previous
"""Causal self-attention for trn2, 8 NeuronCores.

Problem: x[4,2048,1024] @ w_qkv[1024,3072] -> causal MHA (16 heads, d=64)
-> @ w_out[1024,1024].

Sharding: core c handles batch b=c%4 and heads hbase=8*(c//4)..hbase+8
(data parallel on B x tensor parallel on heads). Each core computes the
partial out-projection y_c = att_slice @ w_out[slice]; the host sums the
two partials per batch.

v6: inputs arrive pre-cast to bf16 and pre-arranged on the host (numpy
round-to-nearest, same numerics as the previous on-chip DVE casts), so
the kernel has no f32 weight loads, no SWDGE cast chain, and every
hardware DMA-transpose of x can start at t=0. DMA traffic is split
across the two HWDGE rings (weights on the scalar ring, x-transposes on
the sync ring) so startup is no longer serialized on one ring; y stores
and SBUF-SBUF moves go through gpsimd SWDGE. Attention processes both
heads of a group per kt-pair unit (K=64 score matmuls at PE row-bands
(0,0)/(64,0), exp(h0) overlaps matmuls(h1)); diagonal kt tiles compute
only causally-valid query columns in a packed layout with a uniform
[128,128] triangle affine_select. Softmax denominators ride a ones-row
in the AV matmul; the den row hops to partition 0 via a tiny SWDGE
move, gpsimd partition_broadcast fans it out (custom-DVE ops and the
broadcast only work from partition base 0), then one
reciprocal_approx_fast + multiply per head. PSUM tags: sc0/sc1 (2
banks each), av0/av1 (1 each), proj (2); out-projection reuses sc.
"""

import sys

for p in ("/opt/trn_rl_repo", "/opt/pypackages"):
    if p not in sys.path:
        sys.path.insert(0, p)

import contextlib

import numpy as np

import concourse.bass as bass
import concourse.mybir as mybir
import concourse.tile as tile
from concourse import bacc
from concourse.bass_utils import run_bass_kernel_spmd

F32 = mybir.dt.float32
BF = mybir.dt.bfloat16
EXP = mybir.ActivationFunctionType.Exp

T = 2048          # sequence length
C = 1024          # model dim
HC = 8            # heads per core
D = 64            # head dim
NG = 4            # head-groups of 2 per core
NCT = C // 128    # 8 contraction tiles
NTT = T // 128    # 16 token tiles
SCALE = 0.125     # 1/sqrt(D)

USE_GPSIMD_BCAST = True


def build_nc():
    nc = bacc.Bacc("TRN2", target_bir_lowering=False, debug=False)

    x_d = nc.dram_tensor("x", [T, C], BF, kind="ExternalInput")
    wq_d = nc.dram_tensor("wq", [128, NCT, 512], BF, kind="ExternalInput")
    wk_d = nc.dram_tensor("wk", [128, NCT, 512], BF, kind="ExternalInput")
    wv_d = nc.dram_tensor("wv", [128, NCT, 512], BF, kind="ExternalInput")
    wo_d = nc.dram_tensor("wo", [128, NG, C], BF, kind="ExternalInput")
    y_d = nc.dram_tensor("y", [T, C], F32, kind="ExternalOutput")

    with tile.TileContext(nc) as tc, contextlib.ExitStack() as ctx:
        persist = ctx.enter_context(tc.tile_pool(name="persist", bufs=1))
        work = ctx.enter_context(tc.tile_pool(name="work", bufs=1))
        ps = ctx.enter_context(tc.tile_pool(name="ps", bufs=1, space="PSUM"))
        dpool = ctx.enter_context(tc.tile_pool(name="dram", bufs=1, space="DRAM"))

        kT = [persist.tile([128, T], BF, tag=f"kT{g}", name=f"kT{g}")
              for g in range(NG)]
        V = persist.tile([128, NTT, HC, 65], BF, tag="V")

        # weights: single bf16 loads on the scalar HWDGE ring (the sync
        # ring is busy with the x DMA-transposes at startup)
        wq_bf = persist.tile([128, NCT, 512], BF, tag="wq_bf")
        wk_bf = persist.tile([128, NCT, 512], BF, tag="wk_bf")
        wv_bf = persist.tile([128, NCT, 512], BF, tag="wv_bf")
        wo_bf = persist.tile([128, NG, C], BF, tag="wo_bf")
        for wdram, wbf in ((wq_d, wq_bf), (wk_d, wk_bf), (wv_d, wv_bf),
                           (wo_d, wo_bf)):
            nc.scalar.dma_start(out=wbf, in_=wdram.ap())

        # ones column of V
        ones_f32 = persist.tile([128, NTT, HC], F32, tag="ones")
        nc.vector.memset(ones_f32, 1.0)
        nc.vector.tensor_copy(V[:, :, :, 64], ones_f32)

        for rnd in range(4):
            q0 = rnd * 512  # first token of this quarter
            nkt = 4 * (rnd + 1)

            # ---- xT quarter via hardware DMA-transpose ----
            xTq = [work.tile([128, 512], BF, tag=f"xTq{ct}",
                             name=f"xTq{ct}", bufs=2)
                   for ct in range(NCT)]
            for ct in range(NCT):
                nc.sync.dma_start_transpose(
                    out=xTq[ct],
                    in_=x_d.ap()[q0:q0 + 512, ct * 128:(ct + 1) * 128]
                )

            # ---- qT/kT for this quarter ----
            qTq = []
            for g in range(NG):
                pqk = ps.tile([128, 1024], F32, tag="proj", name="pqk")
                for ct in range(NCT):
                    nc.tensor.matmul(
                        pqk[:, 0:512],
                        wq_bf[:, ct, g * 128:(g + 1) * 128],
                        xTq[ct],
                        start=(ct == 0), stop=(ct == NCT - 1),
                    )
                    nc.tensor.matmul(
                        pqk[:, 512:1024],
                        wk_bf[:, ct, g * 128:(g + 1) * 128],
                        xTq[ct],
                        start=(ct == 0), stop=(ct == NCT - 1),
                    )
                qq = work.tile([128, 512], BF, tag=f"qTq{g}", bufs=2,
                               name=f"qTq{g}")
                nc.vector.tensor_copy(qq, pqk[:, 0:512])
                qTq.append(qq)
                nc.vector.tensor_copy(kT[g][:, q0:q0 + 512], pqk[:, 512:1024])

            # ---- V for this quarter (two tt-pairs per psum tile) ----
            for half in range(2):
                pv = ps.tile([128, 1024], F32, tag="proj", name="pv")
                for ct in range(NCT):
                    for sub in range(2):
                        jl = half * 2 + sub
                        nc.tensor.matmul(
                            pv[:, sub * 512:(sub + 1) * 512],
                            xTq[ct][:, jl * 128:(jl + 1) * 128],
                            wv_bf[:, ct, :],
                            start=(ct == 0), stop=(ct == NCT - 1),
                        )
                tt0 = rnd * 4 + half * 2
                nc.vector.tensor_copy(
                    V[:, tt0:tt0 + 2, :, 0:64],
                    pv.rearrange("p (t h d) -> p t h d", t=2, h=HC),
                )

            # ---- attention: q-block rnd for every group ----
            # Both heads of a group advance together through kt-pair units:
            # the two K=64 score matmuls of a pair pack into PE row-bands
            # 0-63 / 64-127 and run concurrently; exp(h0) overlaps AV(h1).
            # Diagonal kt tiles compute only valid query columns, written
            # at packed offsets so one exp instruction covers them.
            avcs = {}
            for g in range(NG):
                av = [ps.tile([65, 512], F32, tag=f"av{hh}", name=f"av{hh}")
                      for hh in range(2)]
                for b0 in range(0, nkt, 2):
                    jb = b0 - 4 * rnd  # >=0 on the two diagonal units
                    # (m, q-start, wT col offset, ncols, triangle col or None,
                    #  start, stop) per kt of the unit
                    if jb < 0:
                        plan = [(0, 0, 0, 512, None, True, True),
                                (1, 0, 512, 512, None, True, True)]
                        expw = 1024
                    elif jb == 0:
                        plan = [(0, 0, 0, 512, 0, True, True),
                                (1, 128, 512, 384, 512, True, True)]
                        expw = 896
                    else:  # jb == 2: both kts land in psum bank 0 -> one
                        # accumulation group writing disjoint column ranges
                        plan = [(0, 256, 0, 256, 0, True, False),
                                (1, 384, 256, 128, 256, False, True)]
                        expw = 384
                    sc = [ps.tile([128, 1024], F32, tag=f"sc{hh}",
                                  name=f"sc{hh}") for hh in range(2)]
                    for (m, qs, co, ncol, tri, st, sp) in plan:
                        for hh in range(2):
                            r0 = 64 * hh
                            nc.tensor.matmul(
                                sc[hh][:, co:co + ncol],
                                kT[g][r0:r0 + 64,
                                      (b0 + m) * 128:(b0 + m + 1) * 128],
                                qTq[g][r0:r0 + 64, qs:qs + ncol],
                                start=st, stop=sp,
                                tile_position=(r0, 0),
                            )
                    wT = [work.tile([128, 1024], BF, tag=f"wT{hh}", bufs=2,
                                    name=f"wT{hh}") for hh in range(2)]
                    for hh in range(2):
                        nc.scalar.activation(wT[hh][:, 0:expw],
                                             sc[hh][:, 0:expw],
                                             EXP, scale=SCALE)
                    for (m, qs, co, ncol, tri, st, sp) in plan:
                        if tri is None:
                            continue
                        for hh in range(2):
                            nc.gpsimd.affine_select(
                                out=wT[hh][:, tri:tri + 128],
                                in_=wT[hh][:, tri:tri + 128],
                                compare_op=mybir.AluOpType.is_ge,
                                fill=0.0,
                                base=0,
                                pattern=[[1, 128]],
                                channel_multiplier=-1,
                            )
                    for hh in range(2):
                        head = 2 * g + hh
                        for (m, qs, co, ncol, tri, st, sp) in plan:
                            kt = b0 + m
                            nc.tensor.matmul(
                                av[hh][:, qs:qs + ncol],
                                V[:, kt, head, :],
                                wT[hh][:, co:co + ncol],
                                start=(kt == 0), stop=(kt == nkt - 1),
                            )
                # stage AV (with its ones-row denominator) off PSUM
                for hh in range(2):
                    avc = work.tile([65, 512], F32, tag="avc", bufs=8,
                                    name="avc")
                    nc.vector.tensor_copy(avc, av[hh])
                    avcs[(g, hh)] = avc

            attTq = []
            for g in range(NG):
                att = work.tile([128, 512], BF, tag=f"attTq{g}", bufs=2,
                                name=f"attTq{g}")
                for hh in range(2):
                    avc = avcs[(g, hh)]
                    # custom-DVE ops and partition_broadcast only work from
                    # partition base 0, so hop the denominator row from
                    # partition 64 to 0 with a tiny SWDGE SBUF->SBUF move
                    den0 = work.tile([1, 512], F32, tag="den0", bufs=4,
                                     name="den0")
                    if USE_GPSIMD_BCAST:
                        nc.gpsimd.dma_start(out=den0, in_=avc[64:65, :])
                        den_b = work.tile([64, 512], F32, tag="den_b",
                                          bufs=4, name="den_b")
                        nc.gpsimd.partition_broadcast(
                            out_ap=den_b, in_ap=den0)
                    else:
                        den_d = dpool.tile([1, 512], F32, tag="den_d",
                                           bufs=8, name="den_d")
                        nc.sync.dma_start(out=den_d, in_=avc[64:65, :])
                        den_b = work.tile([64, 512], F32, tag="den_b",
                                          bufs=4, name="den_b")
                        nc.sync.dma_start(
                            out=den_b,
                            in_=bass.AP(den_d.tensor, den_d.offset,
                                        [[0, 64], [1, 512]]),
                        )
                    rep = work.tile([64, 512], F32, tag="rep", bufs=4,
                                    name="rep")
                    nc.vector.reciprocal_approx_fast(out=rep, in_=den_b)
                    if hh == 0:
                        nc.vector.tensor_mul(att[0:64, :], avc[0:64, :], rep)
                    else:
                        tmpB = work.tile([64, 512], BF, tag="tmpB", bufs=2,
                                         name="tmpB")
                        nc.vector.tensor_mul(tmpB, avc[0:64, :], rep)
                        nc.gpsimd.dma_start(out=att[64:128, :], in_=tmpB)
                attTq.append(att)

            # ---- out projection for this quarter's q rows ----
            # psum comes from the sc tags (attention is done with them);
            # y stores ride the gpsimd SWDGE to keep the sync ring free
            # for the next round's DMA-transposes
            for qtl in range(4):
                qt = rnd * 4 + qtl
                psy = ps.tile([128, 1024], F32, tag=f"sc{qtl % 2}",
                              name="psy")
                for g in range(NG):
                    for half in range(2):
                        nc.tensor.matmul(
                            psy[:, half * 512:(half + 1) * 512],
                            attTq[g][:, qtl * 128:(qtl + 1) * 128],
                            wo_bf[:, g, half * 512:(half + 1) * 512],
                            start=(g == 0),
                            stop=(g == NG - 1),
                        )
                y_sb = work.tile([128, C], F32, tag="y_sb", bufs=2, name="y_sb")
                nc.vector.tensor_copy(y_sb, psy)
                nc.gpsimd.dma_start(
                    out=y_d.ap()[qt * 128:(qt + 1) * 128, :], in_=y_sb
                )

    nc.compile()
    return nc


_NC_CACHE = None


def _get_nc():
    global _NC_CACHE
    if _NC_CACHE is None:
        _NC_CACHE = build_nc()
    return _NC_CACHE


def kernel(x, w_qkv, w_out, _trace=False):
    import ml_dtypes

    bf16 = ml_dtypes.bfloat16
    B = x.shape[0]
    x = np.asarray(x, dtype=np.float32)
    w_qkv = np.asarray(w_qkv, dtype=np.float32)
    w_out = np.asarray(w_out, dtype=np.float32)

    nc = _get_nc()
    in_maps = []
    for core in range(8):
        b = core % B
        hbase = (core // B) * HC
        lo, hi = hbase * D, hbase * D + HC * D

        def warr(w):  # [C, 512] -> [128, NCT, 512] bf16
            return np.ascontiguousarray(
                w.reshape(NCT, 128, 512).transpose(1, 0, 2).astype(bf16))

        wo = w_out[lo:hi, :]  # [512, C] -> [128, NG, C] bf16
        in_maps.append({
            "x": np.ascontiguousarray(x[b].astype(bf16)),
            "wq": warr(w_qkv[:, lo:hi]),
            "wk": warr(w_qkv[:, C + lo:C + hi]),
            "wv": warr(w_qkv[:, 2 * C + lo:2 * C + hi]),
            "wo": np.ascontiguousarray(
                wo.reshape(NG, 128, C).transpose(1, 0, 2).astype(bf16)),
        })

    res = run_bass_kernel_spmd(nc, in_maps, core_ids=list(range(8)), trace=_trace)
    ys = [r["y"] for r in res.results]
    out = np.empty((B, T, C), dtype=np.float32)
    for b in range(B):
        out[b] = ys[b] + ys[b + B]
    if _trace:
        return out, res
    return out


# revision 20
# speedup vs baseline: 1.4891x; 1.1236x over previous
"""Causal self-attention for trn2, 8 NeuronCores.

Problem: x[4,2048,1024] @ w_qkv[1024,3072] -> causal MHA (16 heads, d=64)
-> @ w_out[1024,1024].

Sharding: core c handles batch b=c%4 and heads hbase=8*(c//4)..hbase+8
(data parallel on B x tensor parallel on heads). Each core computes the
partial out-projection y_c = att_slice @ w_out[slice]; the host sums the
two partials per batch.

v6: inputs arrive pre-cast to bf16 and pre-arranged on the host (numpy
round-to-nearest, same numerics as the previous on-chip DVE casts), so
the kernel has no f32 weight loads, no SWDGE cast chain, and every
hardware DMA-transpose of x can start at t=0. DMA traffic is split
across the two HWDGE rings (weights on the scalar ring, x-transposes on
the sync ring) so startup is no longer serialized on one ring; y stores
and SBUF-SBUF moves go through gpsimd SWDGE. Attention processes both
heads of a group per kt-pair unit (K=64 score matmuls at PE row-bands
(0,0)/(64,0), exp(h0) overlaps matmuls(h1)); diagonal kt tiles compute
only causally-valid query columns in a packed layout with a uniform
[128,128] triangle affine_select. Softmax denominators ride a ones-row
in the AV matmul; the den row hops to partition 0 via a tiny SWDGE
move, gpsimd partition_broadcast fans it out (custom-DVE ops and the
broadcast only work from partition base 0), then one
reciprocal_approx_fast + multiply per head. PSUM tags: sc0/sc1 (2
banks each), av0/av1 (1 each), proj (2); out-projection reuses sc.
"""

import sys

for p in ("/opt/trn_rl_repo", "/opt/pypackages"):
    if p not in sys.path:
        sys.path.insert(0, p)

import contextlib

import numpy as np

import concourse.bass as bass
import concourse.mybir as mybir
import concourse.tile as tile
from concourse import bacc
from concourse.bass_utils import run_bass_kernel_spmd

F32 = mybir.dt.float32
BF = mybir.dt.bfloat16
EXP = mybir.ActivationFunctionType.Exp

T = 2048          # sequence length
C = 1024          # model dim
HC = 8            # heads per core
D = 64            # head dim
NG = 4            # head-groups of 2 per core
NCT = C // 128    # 8 contraction tiles
NTT = T // 128    # 16 token tiles
SCALE = 0.125     # 1/sqrt(D)

USE_GPSIMD_BCAST = True


def build_nc():
    nc = bacc.Bacc("TRN2", target_bir_lowering=False, debug=False)

    x_d = nc.dram_tensor("x", [T, C], BF, kind="ExternalInput")
    # weights ship as two host-packed tensors (2 DMAs, fewer DMA-sem
    # lane conflicts at startup): wqk = [wq | wk], wvo = [wv | wo]
    wqk_d = nc.dram_tensor("wqk", [128, 2 * NCT * 512], BF,
                           kind="ExternalInput")
    wvo_d = nc.dram_tensor("wvo", [128, NCT * 512 + NG * C], BF,
                           kind="ExternalInput")
    y_d = nc.dram_tensor("y", [T, C], F32, kind="ExternalOutput")

    with tile.TileContext(nc) as tc, contextlib.ExitStack() as ctx:
        persist = ctx.enter_context(tc.tile_pool(name="persist", bufs=1))
        work = ctx.enter_context(tc.tile_pool(name="work", bufs=1))
        ps = ctx.enter_context(tc.tile_pool(name="ps", bufs=1, space="PSUM"))
        dpool = ctx.enter_context(tc.tile_pool(name="dram", bufs=1, space="DRAM"))

        kT = [persist.tile([128, T], BF, tag=f"kT{g}", name=f"kT{g}")
              for g in range(NG)]
        V = persist.tile([128, NTT, HC, 65], BF, tag="V")

        # weights: two bf16 loads on the scalar HWDGE ring (the sync
        # ring is busy with the x DMA-transposes at startup)
        wqk_sb = persist.tile([128, 2 * NCT * 512], BF, tag="wqk_sb")
        wvo_sb = persist.tile([128, NCT * 512 + NG * C], BF, tag="wvo_sb")
        nc.scalar.dma_start(out=wqk_sb, in_=wqk_d.ap())
        nc.scalar.dma_start(out=wvo_sb, in_=wvo_d.ap())

        def wq_ap(ct, cols):  # [128, 128-slice of the ct-block]
            return wqk_sb[:, ct * 512 + cols[0]:ct * 512 + cols[1]]

        def wk_ap(ct, cols):
            base = NCT * 512
            return wqk_sb[:, base + ct * 512 + cols[0]:base + ct * 512 + cols[1]]

        def wv_ap(ct):
            return wvo_sb[:, ct * 512:(ct + 1) * 512]

        def wo_ap(g, cols):
            base = NCT * 512
            return wvo_sb[:, base + g * C + cols[0]:base + g * C + cols[1]]

        # ones column of V
        ones_f32 = persist.tile([128, NTT, HC], F32, tag="ones")
        nc.vector.memset(ones_f32, 1.0)
        nc.vector.tensor_copy(V[:, :, :, 64], ones_f32)

        for rnd in range(4):
            q0 = rnd * 512  # first token of this quarter
            nkt = 4 * (rnd + 1)

            # ---- xT quarter via hardware DMA-transpose ----
            xTq = [work.tile([128, 512], BF, tag=f"xTq{ct}",
                             name=f"xTq{ct}", bufs=2)
                   for ct in range(NCT)]
            for ct in range(NCT):
                nc.sync.dma_start_transpose(
                    out=xTq[ct],
                    in_=x_d.ap()[q0:q0 + 512, ct * 128:(ct + 1) * 128]
                )

            # ---- qT/kT for this quarter ----
            qTq = []
            for g in range(NG):
                pqk = ps.tile([128, 1024], F32, tag="proj", name="pqk")
                # q-chain fully before k-chain so the PE isn't stalled on
                # the wk half of the weight load at startup
                for ct in range(NCT):
                    nc.tensor.matmul(
                        pqk[:, 0:512],
                        wq_ap(ct, (g * 128, (g + 1) * 128)),
                        xTq[ct],
                        start=(ct == 0), stop=(ct == NCT - 1),
                    )
                for ct in range(NCT):
                    nc.tensor.matmul(
                        pqk[:, 512:1024],
                        wk_ap(ct, (g * 128, (g + 1) * 128)),
                        xTq[ct],
                        start=(ct == 0), stop=(ct == NCT - 1),
                    )
                qq = work.tile([128, 512], BF, tag=f"qTq{g}", bufs=2,
                               name=f"qTq{g}")
                nc.vector.tensor_copy(qq, pqk[:, 0:512])
                qTq.append(qq)
                nc.vector.tensor_copy(kT[g][:, q0:q0 + 512], pqk[:, 512:1024])

            # ---- V for this quarter (two tt-pairs per psum tile) ----
            for half in range(2):
                pv = ps.tile([128, 1024], F32, tag="proj", name="pv")
                for ct in range(NCT):
                    for sub in range(2):
                        jl = half * 2 + sub
                        nc.tensor.matmul(
                            pv[:, sub * 512:(sub + 1) * 512],
                            xTq[ct][:, jl * 128:(jl + 1) * 128],
                            wv_ap(ct),
                            start=(ct == 0), stop=(ct == NCT - 1),
                        )
                tt0 = rnd * 4 + half * 2
                nc.vector.tensor_copy(
                    V[:, tt0:tt0 + 2, :, 0:64],
                    pv.rearrange("p (t h d) -> p t h d", t=2, h=HC),
                )

            # ---- attention: q-block rnd for every group ----
            # Both heads of a group advance together through kt-pair units:
            # the two K=64 score matmuls of a pair pack into PE row-bands
            # 0-63 / 64-127 and run concurrently; exp(h0) overlaps AV(h1).
            # Diagonal kt tiles compute only valid query columns, written
            # at packed offsets so one exp instruction covers them.
            attTq = []
            for g in range(NG):
                av = [ps.tile([65, 512], F32, tag=f"av{hh}", name=f"av{hh}")
                      for hh in range(2)]
                for b0 in range(0, nkt, 2):
                    jb = b0 - 4 * rnd  # >=0 on the two diagonal units
                    # (m, q-start, wT col offset, ncols, triangle col or None,
                    #  start, stop) per kt of the unit
                    if jb < 0:
                        plan = [(0, 0, 0, 512, None, True, True),
                                (1, 0, 512, 512, None, True, True)]
                        expw = 1024
                    elif jb == 0:
                        plan = [(0, 0, 0, 512, 0, True, True),
                                (1, 128, 512, 384, 512, True, True)]
                        expw = 896
                    else:  # jb == 2: both kts land in psum bank 0 -> one
                        # accumulation group writing disjoint column ranges
                        plan = [(0, 256, 0, 256, 0, True, False),
                                (1, 384, 256, 128, 256, False, True)]
                        expw = 384
                    sc = [ps.tile([128, 1024], F32, tag=f"sc{hh}",
                                  name=f"sc{hh}") for hh in range(2)]
                    for (m, qs, co, ncol, tri, st, sp) in plan:
                        for hh in range(2):
                            r0 = 64 * hh
                            nc.tensor.matmul(
                                sc[hh][:, co:co + ncol],
                                kT[g][r0:r0 + 64,
                                      (b0 + m) * 128:(b0 + m + 1) * 128],
                                qTq[g][r0:r0 + 64, qs:qs + ncol],
                                start=st, stop=sp,
                                tile_position=(r0, 0),
                            )
                    wT = [work.tile([128, 1024], BF, tag=f"wT{hh}", bufs=2,
                                    name=f"wT{hh}") for hh in range(2)]
                    for hh in range(2):
                        nc.scalar.activation(wT[hh][:, 0:expw],
                                             sc[hh][:, 0:expw],
                                             EXP, scale=SCALE)
                    for (m, qs, co, ncol, tri, st, sp) in plan:
                        if tri is None:
                            continue
                        for hh in range(2):
                            nc.gpsimd.affine_select(
                                out=wT[hh][:, tri:tri + 128],
                                in_=wT[hh][:, tri:tri + 128],
                                compare_op=mybir.AluOpType.is_ge,
                                fill=0.0,
                                base=0,
                                pattern=[[1, 128]],
                                channel_multiplier=-1,
                            )
                    for hh in range(2):
                        head = 2 * g + hh
                        for (m, qs, co, ncol, tri, st, sp) in plan:
                            kt = b0 + m
                            nc.tensor.matmul(
                                av[hh][:, qs:qs + ncol],
                                V[:, kt, head, :],
                                wT[hh][:, co:co + ncol],
                                start=(kt == 0), stop=(kt == nkt - 1),
                            )
                # stage AV off PSUM and normalize this group right away
                # (keeps the av/avc rotations short and spreads the
                # normalization work across the round)
                att = work.tile([128, 512], BF, tag=f"attTq{g}", bufs=2,
                                name=f"attTq{g}")
                for hh in range(2):
                    avc = work.tile([65, 512], F32, tag="avc", bufs=4,
                                    name="avc")
                    nc.vector.tensor_copy(avc, av[hh])
                    # custom-DVE ops and partition_broadcast only work from
                    # partition base 0, so hop the denominator row from
                    # partition 64 to 0 (HWDGE SBUF->SBUF on the sync ring)
                    den0 = work.tile([1, 512], F32, tag="den0", bufs=4,
                                     name="den0")
                    nc.sync.dma_start(out=den0, in_=avc[64:65, :])
                    den_b = work.tile([64, 512], F32, tag="den_b",
                                      bufs=4, name="den_b")
                    nc.gpsimd.partition_broadcast(out_ap=den_b, in_ap=den0)
                    rep = work.tile([64, 512], F32, tag="rep", bufs=4,
                                    name="rep")
                    nc.vector.reciprocal_approx_fast(out=rep, in_=den_b)
                    if hh == 0:
                        nc.vector.tensor_mul(att[0:64, :], avc[0:64, :], rep)
                    else:
                        tmpB = work.tile([64, 512], BF, tag="tmpB", bufs=2,
                                         name="tmpB")
                        nc.vector.tensor_mul(tmpB, avc[0:64, :], rep)
                        nc.gpsimd.dma_start(out=att[64:128, :], in_=tmpB)
                attTq.append(att)

            # ---- out projection for this quarter's q rows ----
            # psum comes from the sc tags (attention is done with them);
            # y stores ride the gpsimd SWDGE to keep the sync ring free
            # for the next round's DMA-transposes
            for qtl in range(4):
                qt = rnd * 4 + qtl
                psy = ps.tile([128, 1024], F32, tag=f"sc{qtl % 2}",
                              name="psy")
                for g in range(NG):
                    for half in range(2):
                        nc.tensor.matmul(
                            psy[:, half * 512:(half + 1) * 512],
                            attTq[g][:, qtl * 128:(qtl + 1) * 128],
                            wo_ap(g, (half * 512, (half + 1) * 512)),
                            start=(g == 0),
                            stop=(g == NG - 1),
                        )
                y_sb = work.tile([128, C], F32, tag="y_sb", bufs=2, name="y_sb")
                nc.vector.tensor_copy(y_sb, psy)
                nc.gpsimd.dma_start(
                    out=y_d.ap()[qt * 128:(qt + 1) * 128, :], in_=y_sb
                )

    nc.compile()
    return nc


_NC_CACHE = None


def _get_nc():
    global _NC_CACHE
    if _NC_CACHE is None:
        _NC_CACHE = build_nc()
    return _NC_CACHE


def kernel(x, w_qkv, w_out, _trace=False):
    import ml_dtypes

    bf16 = ml_dtypes.bfloat16
    B = x.shape[0]
    x = np.asarray(x, dtype=np.float32)
    w_qkv = np.asarray(w_qkv, dtype=np.float32)
    w_out = np.asarray(w_out, dtype=np.float32)

    nc = _get_nc()
    in_maps = []
    for core in range(8):
        b = core % B
        hbase = (core // B) * HC
        lo, hi = hbase * D, hbase * D + HC * D

        def warr(w):  # [C, 512] -> [128, NCT*512]
            return w.reshape(NCT, 128, 512).transpose(1, 0, 2).reshape(
                128, NCT * 512)

        wo = w_out[lo:hi, :].reshape(NG, 128, C).transpose(1, 0, 2).reshape(
            128, NG * C)  # [512, C] -> [128, NG*C]
        wqk = np.concatenate(
            [warr(w_qkv[:, lo:hi]), warr(w_qkv[:, C + lo:C + hi])], axis=1)
        wvo = np.concatenate(
            [warr(w_qkv[:, 2 * C + lo:2 * C + hi]), wo], axis=1)
        in_maps.append({
            "x": np.ascontiguousarray(x[b].astype(bf16)),
            "wqk": np.ascontiguousarray(wqk.astype(bf16)),
            "wvo": np.ascontiguousarray(wvo.astype(bf16)),
        })

    res = run_bass_kernel_spmd(nc, in_maps, core_ids=list(range(8)), trace=_trace)
    ys = [r["y"] for r in res.results]
    out = np.empty((B, T, C), dtype=np.float32)
    for b in range(B):
        out[b] = ys[b] + ys[b + B]
    if _trace:
        return out, res
    return out


# revision 23
# speedup vs baseline: 1.5087x; 1.0131x over previous
"""Causal self-attention for trn2, 8 NeuronCores.

Problem: x[4,2048,1024] @ w_qkv[1024,3072] -> causal MHA (16 heads, d=64)
-> @ w_out[1024,1024].

Sharding: core c handles batch b=c%4 and heads hbase=8*(c//4)..hbase+8
(data parallel on B x tensor parallel on heads). Each core computes the
partial out-projection y_c = att_slice @ w_out[slice]; the host sums the
two partials per batch.

v6: inputs arrive pre-cast to bf16 and pre-arranged on the host (numpy
round-to-nearest, same numerics as the previous on-chip DVE casts), so
the kernel has no f32 weight loads, no SWDGE cast chain, and every
hardware DMA-transpose of x can start at t=0. DMA traffic is split
across the two HWDGE rings (weights on the scalar ring, x-transposes on
the sync ring) so startup is no longer serialized on one ring; y stores
and SBUF-SBUF moves go through gpsimd SWDGE. Attention processes both
heads of a group per kt-pair unit (K=64 score matmuls at PE row-bands
(0,0)/(64,0), exp(h0) overlaps matmuls(h1)); diagonal kt tiles compute
only causally-valid query columns in a packed layout with a uniform
[128,128] triangle affine_select. Softmax denominators ride a ones-row
in the AV matmul; the den row hops to partition 0 via a tiny SWDGE
move, gpsimd partition_broadcast fans it out (custom-DVE ops and the
broadcast only work from partition base 0), then one
reciprocal_approx_fast + multiply per head. PSUM tags: sc0/sc1 (2
banks each), av0/av1 (1 each), proj (2); out-projection reuses sc.
"""

import sys

for p in ("/opt/trn_rl_repo", "/opt/pypackages"):
    if p not in sys.path:
        sys.path.insert(0, p)

import contextlib

import numpy as np

import concourse.bass as bass
import concourse.mybir as mybir
import concourse.tile as tile
from concourse import bacc
from concourse.bass_utils import run_bass_kernel_spmd

F32 = mybir.dt.float32
BF = mybir.dt.bfloat16
EXP = mybir.ActivationFunctionType.Exp

T = 2048          # sequence length
C = 1024          # model dim
HC = 8            # heads per core
D = 64            # head dim
NG = 4            # head-groups of 2 per core
NCT = C // 128    # 8 contraction tiles
NTT = T // 128    # 16 token tiles
SCALE = 0.125     # 1/sqrt(D)

USE_GPSIMD_BCAST = True


def build_nc():
    nc = bacc.Bacc("TRN2", target_bir_lowering=False, debug=False)

    x_d = nc.dram_tensor("x", [T, C], BF, kind="ExternalInput")
    # weights ship as two host-packed tensors (2 DMAs, fewer DMA-sem
    # lane conflicts at startup): wqk = [wq | wk], wvo = [wv | wo]
    wqk_d = nc.dram_tensor("wqk", [128, 2 * NCT * 512], BF,
                           kind="ExternalInput")
    wvo_d = nc.dram_tensor("wvo", [128, NCT * 512 + NG * C], BF,
                           kind="ExternalInput")
    y_d = nc.dram_tensor("y", [T, C], F32, kind="ExternalOutput")

    with tile.TileContext(nc) as tc, contextlib.ExitStack() as ctx:
        persist = ctx.enter_context(tc.tile_pool(name="persist", bufs=1))
        work = ctx.enter_context(tc.tile_pool(name="work", bufs=1))
        ps = ctx.enter_context(tc.tile_pool(name="ps", bufs=1, space="PSUM"))
        dpool = ctx.enter_context(tc.tile_pool(name="dram", bufs=1, space="DRAM"))

        kT = [persist.tile([128, T], BF, tag=f"kT{g}", name=f"kT{g}")
              for g in range(NG)]
        V = persist.tile([128, NTT, HC, 65], BF, tag="V")

        # weights: two bf16 loads on the scalar HWDGE ring (the sync
        # ring is busy with the x DMA-transposes at startup)
        wqk_sb = persist.tile([128, 2 * NCT * 512], BF, tag="wqk_sb")
        wvo_sb = persist.tile([128, NCT * 512 + NG * C], BF, tag="wvo_sb")
        nc.scalar.dma_start(out=wqk_sb, in_=wqk_d.ap())
        nc.scalar.dma_start(out=wvo_sb, in_=wvo_d.ap())

        def wq_ap(ct, cols):  # [128, 128-slice of the ct-block]
            return wqk_sb[:, ct * 512 + cols[0]:ct * 512 + cols[1]]

        def wk_ap(ct, cols):
            base = NCT * 512
            return wqk_sb[:, base + ct * 512 + cols[0]:base + ct * 512 + cols[1]]

        def wv_ap(ct):
            return wvo_sb[:, ct * 512:(ct + 1) * 512]

        def wo_ap(g, cols):
            base = NCT * 512
            return wvo_sb[:, base + g * C + cols[0]:base + g * C + cols[1]]

        # ones column of V
        ones_f32 = persist.tile([128, NTT, HC], F32, tag="ones")
        nc.vector.memset(ones_f32, 1.0)
        nc.vector.tensor_copy(V[:, :, :, 64], ones_f32)

        for rnd in range(4):
            q0 = rnd * 512  # first token of this quarter
            nkt = 4 * (rnd + 1)

            # ---- xT quarter via hardware DMA-transpose ----
            xTq = [work.tile([128, 512], BF, tag=f"xTq{ct}",
                             name=f"xTq{ct}", bufs=2)
                   for ct in range(NCT)]
            for ct in range(NCT):
                nc.sync.dma_start_transpose(
                    out=xTq[ct],
                    in_=x_d.ap()[q0:q0 + 512, ct * 128:(ct + 1) * 128]
                )

            # ---- qT/kT for this quarter ----
            qTq = []
            for g in range(NG):
                pqk = ps.tile([128, 1024], F32, tag="proj", name="pqk")
                # q-chain fully before k-chain so the PE isn't stalled on
                # the wk half of the weight load at startup
                for ct in range(NCT):
                    nc.tensor.matmul(
                        pqk[:, 0:512],
                        wq_ap(ct, (g * 128, (g + 1) * 128)),
                        xTq[ct],
                        start=(ct == 0), stop=(ct == NCT - 1),
                    )
                for ct in range(NCT):
                    nc.tensor.matmul(
                        pqk[:, 512:1024],
                        wk_ap(ct, (g * 128, (g + 1) * 128)),
                        xTq[ct],
                        start=(ct == 0), stop=(ct == NCT - 1),
                    )
                qq = work.tile([128, 512], BF, tag=f"qTq{g}", bufs=2,
                               name=f"qTq{g}")
                nc.vector.tensor_copy(qq, pqk[:, 0:512])
                qTq.append(qq)
                nc.vector.tensor_copy(kT[g][:, q0:q0 + 512], pqk[:, 512:1024])

            # ---- V for this quarter (two tt-pairs per psum tile) ----
            for half in range(2):
                pv = ps.tile([128, 1024], F32, tag="proj", name="pv")
                for ct in range(NCT):
                    for sub in range(2):
                        jl = half * 2 + sub
                        nc.tensor.matmul(
                            pv[:, sub * 512:(sub + 1) * 512],
                            xTq[ct][:, jl * 128:(jl + 1) * 128],
                            wv_ap(ct),
                            start=(ct == 0), stop=(ct == NCT - 1),
                        )
                tt0 = rnd * 4 + half * 2
                nc.vector.tensor_copy(
                    V[:, tt0:tt0 + 2, :, 0:64],
                    pv.rearrange("p (t h d) -> p t h d", t=2, h=HC),
                )

            # ---- attention: q-block rnd for every group ----
            # Both heads of a group advance together through kt-pair units:
            # the two K=64 score matmuls of a pair pack into PE row-bands
            # 0-63 / 64-127 and run concurrently; exp(h0) overlaps AV(h1).
            # Diagonal kt tiles compute only valid query columns, written
            # at packed offsets so one exp instruction covers them.
            attTq = []
            for g in range(NG):
                av = [ps.tile([65, 512], F32, tag=f"av{hh}", name=f"av{hh}")
                      for hh in range(2)]

                # full (below-diagonal) kt tiles: one kt per step, BOTH
                # heads' scores in one [128,1024] psum tile. The two K=64
                # score matmuls are simultaneously ready and adjacent, so
                # they pack into PE row-bands and run concurrently; one
                # exp covers both heads. AV(kt-1) is emitted after
                # scores/exp(kt) (software pipeline skew) so the PE queue
                # never head-blocks on an exp in flight.
                pend = None
                for kt in range(4 * rnd):
                    sc = ps.tile([128, 1024], F32, tag="sc", bufs=2,
                                 name="sc")
                    for hh in range(2):
                        r0 = 64 * hh
                        nc.tensor.matmul(
                            sc[:, hh * 512:(hh + 1) * 512],
                            kT[g][r0:r0 + 64, kt * 128:(kt + 1) * 128],
                            qTq[g][r0:r0 + 64, :],
                            start=True, stop=True,
                            tile_position=(r0, 0),
                        )
                    wT = work.tile([128, 1024], BF, tag="wT", bufs=3,
                                   name="wT")
                    nc.scalar.activation(wT, sc, EXP, scale=SCALE)
                    if pend is not None:
                        pkt, pwT = pend
                        for hh in range(2):
                            nc.tensor.matmul(
                                av[hh][:, 0:512],
                                V[:, pkt, 2 * g + hh, :],
                                pwT[:, hh * 512:(hh + 1) * 512],
                                start=(pkt == 0), stop=False,
                            )
                    pend = (kt, wT)
                if pend is not None:
                    pkt, pwT = pend
                    for hh in range(2):
                        nc.tensor.matmul(
                            av[hh][:, 0:512],
                            V[:, pkt, 2 * g + hh, :],
                            pwT[:, hh * 512:(hh + 1) * 512],
                            start=(pkt == 0), stop=False,
                        )

                # the four diagonal kt tiles, as two kt-pair units with
                # causally-restricted packed columns (per-head sc tiles
                # drawn from the same 2-buffer sc tag)
                for b0 in (4 * rnd, 4 * rnd + 2):
                    jb = b0 - 4 * rnd
                    # (m, q-start, wT col offset, ncols, triangle col or
                    #  None, start, stop) per kt of the unit
                    if jb == 0:
                        plan = [(0, 0, 0, 512, 0, True, True),
                                (1, 128, 512, 384, 512, True, True)]
                        expw = 896
                    else:  # jb == 2: both kts land in psum bank 0 -> one
                        # accumulation group writing disjoint column ranges
                        plan = [(0, 256, 0, 256, 0, True, False),
                                (1, 384, 256, 128, 256, False, True)]
                        expw = 384
                    sc2 = [ps.tile([128, 1024], F32, tag="sc", bufs=2,
                                   name=f"sc{hh}") for hh in range(2)]
                    for (m, qs, co, ncol, tri, st, sp) in plan:
                        for hh in range(2):
                            r0 = 64 * hh
                            nc.tensor.matmul(
                                sc2[hh][:, co:co + ncol],
                                kT[g][r0:r0 + 64,
                                      (b0 + m) * 128:(b0 + m + 1) * 128],
                                qTq[g][r0:r0 + 64, qs:qs + ncol],
                                start=st, stop=sp,
                                tile_position=(r0, 0),
                            )
                    wT2 = [work.tile([128, 1024], BF, tag="wT", bufs=3,
                                     name=f"wTd{hh}") for hh in range(2)]
                    for hh in range(2):
                        nc.scalar.activation(wT2[hh][:, 0:expw],
                                             sc2[hh][:, 0:expw],
                                             EXP, scale=SCALE)
                    for (m, qs, co, ncol, tri, st, sp) in plan:
                        if tri is None:
                            continue
                        for hh in range(2):
                            nc.gpsimd.affine_select(
                                out=wT2[hh][:, tri:tri + 128],
                                in_=wT2[hh][:, tri:tri + 128],
                                compare_op=mybir.AluOpType.is_ge,
                                fill=0.0,
                                base=0,
                                pattern=[[1, 128]],
                                channel_multiplier=-1,
                            )
                    for hh in range(2):
                        head = 2 * g + hh
                        for (m, qs, co, ncol, tri, st, sp) in plan:
                            kt = b0 + m
                            nc.tensor.matmul(
                                av[hh][:, qs:qs + ncol],
                                V[:, kt, head, :],
                                wT2[hh][:, co:co + ncol],
                                start=(kt == 0), stop=(kt == nkt - 1),
                            )
                # stage AV off PSUM and normalize this group right away
                # (keeps the av/avc rotations short and spreads the
                # normalization work across the round)
                att = work.tile([128, 512], BF, tag=f"attTq{g}", bufs=2,
                                name=f"attTq{g}")
                for hh in range(2):
                    avc = work.tile([65, 512], F32, tag="avc", bufs=4,
                                    name="avc")
                    nc.vector.tensor_copy(avc, av[hh])
                    # custom-DVE ops and partition_broadcast only work from
                    # partition base 0, so hop the denominator row from
                    # partition 64 to 0 (HWDGE SBUF->SBUF on the sync ring)
                    den0 = work.tile([1, 512], F32, tag="den0", bufs=4,
                                     name="den0")
                    nc.sync.dma_start(out=den0, in_=avc[64:65, :])
                    den_b = work.tile([64, 512], F32, tag="den_b",
                                      bufs=4, name="den_b")
                    nc.gpsimd.partition_broadcast(out_ap=den_b, in_ap=den0)
                    rep = work.tile([64, 512], F32, tag="rep", bufs=4,
                                    name="rep")
                    nc.vector.reciprocal_approx_fast(out=rep, in_=den_b)
                    if hh == 0:
                        nc.vector.tensor_mul(att[0:64, :], avc[0:64, :], rep)
                    else:
                        tmpB = work.tile([64, 512], BF, tag="tmpB", bufs=2,
                                         name="tmpB")
                        nc.vector.tensor_mul(tmpB, avc[0:64, :], rep)
                        nc.gpsimd.dma_start(out=att[64:128, :], in_=tmpB)
                attTq.append(att)

            # ---- out projection for this quarter's q rows ----
            # psum comes from the sc tags (attention is done with them);
            # y stores ride the gpsimd SWDGE to keep the sync ring free
            # for the next round's DMA-transposes
            for qtl in range(4):
                qt = rnd * 4 + qtl
                psy = ps.tile([128, 1024], F32, tag="sc", bufs=2,
                              name="psy")
                for g in range(NG):
                    for half in range(2):
                        nc.tensor.matmul(
                            psy[:, half * 512:(half + 1) * 512],
                            attTq[g][:, qtl * 128:(qtl + 1) * 128],
                            wo_ap(g, (half * 512, (half + 1) * 512)),
                            start=(g == 0),
                            stop=(g == NG - 1),
                        )
                y_sb = work.tile([128, C], F32, tag="y_sb", bufs=2, name="y_sb")
                nc.vector.tensor_copy(y_sb, psy)
                nc.gpsimd.dma_start(
                    out=y_d.ap()[qt * 128:(qt + 1) * 128, :], in_=y_sb
                )

    nc.compile()
    return nc


_NC_CACHE = None


def _get_nc():
    global _NC_CACHE
    if _NC_CACHE is None:
        _NC_CACHE = build_nc()
    return _NC_CACHE


def kernel(x, w_qkv, w_out, _trace=False):
    import ml_dtypes

    bf16 = ml_dtypes.bfloat16
    B = x.shape[0]
    x = np.asarray(x, dtype=np.float32)
    w_qkv = np.asarray(w_qkv, dtype=np.float32)
    w_out = np.asarray(w_out, dtype=np.float32)

    nc = _get_nc()
    in_maps = []
    for core in range(8):
        b = core % B
        hbase = (core // B) * HC
        lo, hi = hbase * D, hbase * D + HC * D

        def warr(w):  # [C, 512] -> [128, NCT*512]
            return w.reshape(NCT, 128, 512).transpose(1, 0, 2).reshape(
                128, NCT * 512)

        wo = w_out[lo:hi, :].reshape(NG, 128, C).transpose(1, 0, 2).reshape(
            128, NG * C)  # [512, C] -> [128, NG*C]
        wqk = np.concatenate(
            [warr(w_qkv[:, lo:hi]), warr(w_qkv[:, C + lo:C + hi])], axis=1)
        wvo = np.concatenate(
            [warr(w_qkv[:, 2 * C + lo:2 * C + hi]), wo], axis=1)
        in_maps.append({
            "x": np.ascontiguousarray(x[b].astype(bf16)),
            "wqk": np.ascontiguousarray(wqk.astype(bf16)),
            "wvo": np.ascontiguousarray(wvo.astype(bf16)),
        })

    res = run_bass_kernel_spmd(nc, in_maps, core_ids=list(range(8)), trace=_trace)
    ys = [r["y"] for r in res.results]
    out = np.empty((B, T, C), dtype=np.float32)
    for b in range(B):
        out[b] = ys[b] + ys[b + B]
    if _trace:
        return out, res
    return out


# revision 26
# speedup vs baseline: 1.5253x; 1.0111x over previous
"""Causal self-attention for trn2, 8 NeuronCores.

Problem: x[4,2048,1024] @ w_qkv[1024,3072] -> causal MHA (16 heads, d=64)
-> @ w_out[1024,1024].

Sharding: core c handles batch b=c%4 and heads hbase=8*(c//4)..hbase+8
(data parallel on B x tensor parallel on heads). Each core computes the
partial out-projection y_c = att_slice @ w_out[slice]; the host sums the
two partials per batch.

v6: inputs arrive pre-cast to bf16 and pre-arranged on the host (numpy
round-to-nearest, same numerics as the previous on-chip DVE casts), so
the kernel has no f32 weight loads, no SWDGE cast chain, and every
hardware DMA-transpose of x can start at t=0. DMA traffic is split
across the two HWDGE rings (weights on the scalar ring, x-transposes on
the sync ring) so startup is no longer serialized on one ring; y stores
and SBUF-SBUF moves go through gpsimd SWDGE. Attention processes both
heads of a group per kt-pair unit (K=64 score matmuls at PE row-bands
(0,0)/(64,0), exp(h0) overlaps matmuls(h1)); diagonal kt tiles compute
only causally-valid query columns in a packed layout with a uniform
[128,128] triangle affine_select. Softmax denominators ride a ones-row
in the AV matmul; the den row hops to partition 0 via a tiny SWDGE
move, gpsimd partition_broadcast fans it out (custom-DVE ops and the
broadcast only work from partition base 0), then one
reciprocal_approx_fast + multiply per head. PSUM tags: sc0/sc1 (2
banks each), av0/av1 (1 each), proj (2); out-projection reuses sc.
"""

import sys

for p in ("/opt/trn_rl_repo", "/opt/pypackages"):
    if p not in sys.path:
        sys.path.insert(0, p)

import contextlib

import numpy as np

import concourse.bass as bass
import concourse.mybir as mybir
import concourse.tile as tile
from concourse import bacc
from concourse.bass_utils import run_bass_kernel_spmd

F32 = mybir.dt.float32
BF = mybir.dt.bfloat16
EXP = mybir.ActivationFunctionType.Exp

T = 2048          # sequence length
C = 1024          # model dim
HC = 8            # heads per core
D = 64            # head dim
NG = 4            # head-groups of 2 per core
NCT = C // 128    # 8 contraction tiles
NTT = T // 128    # 16 token tiles
SCALE = 0.125     # 1/sqrt(D)

USE_GPSIMD_BCAST = True


def build_nc():
    nc = bacc.Bacc("TRN2", target_bir_lowering=False, debug=False)

    x_d = nc.dram_tensor("x", [T, C], BF, kind="ExternalInput")
    # weights ship as two host-packed tensors (2 DMAs, fewer DMA-sem
    # lane conflicts at startup): wqk = [wq | wk], wvo = [wv | wo]
    wqk_d = nc.dram_tensor("wqk", [128, 2 * NCT * 512], BF,
                           kind="ExternalInput")
    wvo_d = nc.dram_tensor("wvo", [128, NCT * 512 + NG * C], BF,
                           kind="ExternalInput")
    y_d = nc.dram_tensor("y", [T, C], F32, kind="ExternalOutput")

    with tile.TileContext(nc) as tc, contextlib.ExitStack() as ctx:
        persist = ctx.enter_context(tc.tile_pool(name="persist", bufs=1))
        work = ctx.enter_context(tc.tile_pool(name="work", bufs=1))
        ps = ctx.enter_context(tc.tile_pool(name="ps", bufs=1, space="PSUM"))
        dpool = ctx.enter_context(tc.tile_pool(name="dram", bufs=1, space="DRAM"))

        kT = [persist.tile([128, T], BF, tag=f"kT{g}", name=f"kT{g}")
              for g in range(NG)]
        V = persist.tile([128, NTT, HC, 65], BF, tag="V")

        # weights: two bf16 loads on the scalar HWDGE ring (the sync
        # ring is busy with the x DMA-transposes at startup)
        wqk_sb = persist.tile([128, 2 * NCT * 512], BF, tag="wqk_sb")
        wvo_sb = persist.tile([128, NCT * 512 + NG * C], BF, tag="wvo_sb")
        nc.scalar.dma_start(out=wqk_sb, in_=wqk_d.ap())
        nc.scalar.dma_start(out=wvo_sb, in_=wvo_d.ap())

        def wq_ap(ct, cols):  # [128, 128-slice of the ct-block]
            return wqk_sb[:, ct * 512 + cols[0]:ct * 512 + cols[1]]

        def wk_ap(ct, cols):
            base = NCT * 512
            return wqk_sb[:, base + ct * 512 + cols[0]:base + ct * 512 + cols[1]]

        def wv_ap(ct):
            return wvo_sb[:, ct * 512:(ct + 1) * 512]

        def wo_ap(g, cols):
            base = NCT * 512
            return wvo_sb[:, base + g * C + cols[0]:base + g * C + cols[1]]

        # ones column of V
        ones_f32 = persist.tile([128, NTT, HC], F32, tag="ones")
        nc.vector.memset(ones_f32, 1.0)
        nc.vector.tensor_copy(V[:, :, :, 64], ones_f32)

        def emit_out_proj(rnd_, att_):
            # out projection of round rnd_'s q rows (psum from the proj
            # tag; y stores ride the gpsimd SWDGE so the sync ring stays
            # free for DMA-transposes and denominator hops)
            for qtl in range(4):
                qt = rnd_ * 4 + qtl
                psy = ps.tile([128, 1024], F32, tag="proj", name="psy")
                for g in range(NG):
                    for half in range(2):
                        nc.tensor.matmul(
                            psy[:, half * 512:(half + 1) * 512],
                            att_[g][:, qtl * 128:(qtl + 1) * 128],
                            wo_ap(g, (half * 512, (half + 1) * 512)),
                            start=(g == 0),
                            stop=(g == NG - 1),
                        )
                y_sb = work.tile([128, C], F32, tag="y_sb", bufs=2,
                                 name="y_sb")
                nc.vector.tensor_copy(y_sb, psy)
                nc.gpsimd.dma_start(
                    out=y_d.ap()[qt * 128:(qt + 1) * 128, :], in_=y_sb
                )

        prev_attTq = None
        for rnd in range(4):
            q0 = rnd * 512  # first token of this quarter
            nkt = 4 * (rnd + 1)

            # ---- xT quarter via hardware DMA-transpose ----
            xTq = [work.tile([128, 512], BF, tag=f"xTq{ct}",
                             name=f"xTq{ct}", bufs=2)
                   for ct in range(NCT)]
            for ct in range(NCT):
                nc.sync.dma_start_transpose(
                    out=xTq[ct],
                    in_=x_d.ap()[q0:q0 + 512, ct * 128:(ct + 1) * 128]
                )

            # ---- qT/kT for this quarter ----
            qTq = []
            for g in range(NG):
                pqk = ps.tile([128, 1024], F32, tag="proj", name="pqk")
                # q-chain fully before k-chain so the PE isn't stalled on
                # the wk half of the weight load at startup
                for ct in range(NCT):
                    nc.tensor.matmul(
                        pqk[:, 0:512],
                        wq_ap(ct, (g * 128, (g + 1) * 128)),
                        xTq[ct],
                        start=(ct == 0), stop=(ct == NCT - 1),
                    )
                for ct in range(NCT):
                    nc.tensor.matmul(
                        pqk[:, 512:1024],
                        wk_ap(ct, (g * 128, (g + 1) * 128)),
                        xTq[ct],
                        start=(ct == 0), stop=(ct == NCT - 1),
                    )
                qq = work.tile([128, 512], BF, tag=f"qTq{g}", bufs=2,
                               name=f"qTq{g}")
                nc.vector.tensor_copy(qq, pqk[:, 0:512])
                qTq.append(qq)
                nc.vector.tensor_copy(kT[g][:, q0:q0 + 512], pqk[:, 512:1024])

            # ---- V for this quarter (two tt-pairs per psum tile) ----
            for half in range(2):
                pv = ps.tile([128, 1024], F32, tag="proj", name="pv")
                for ct in range(NCT):
                    for sub in range(2):
                        jl = half * 2 + sub
                        nc.tensor.matmul(
                            pv[:, sub * 512:(sub + 1) * 512],
                            xTq[ct][:, jl * 128:(jl + 1) * 128],
                            wv_ap(ct),
                            start=(ct == 0), stop=(ct == NCT - 1),
                        )
                tt0 = rnd * 4 + half * 2
                nc.vector.tensor_copy(
                    V[:, tt0:tt0 + 2, :, 0:64],
                    pv.rearrange("p (t h d) -> p t h d", t=2, h=HC),
                )

            # ---- deferred out-projection of the PREVIOUS round ----
            # Emitted here (after this round's qk/V, before its attention)
            # so the proj-tag rotation lets it overlap this round's
            # attention, and the sc tag stays attention-only: the next
            # round's scores never wait on out-proj psum.
            if rnd > 0:
                emit_out_proj(rnd - 1, prev_attTq)

            # ---- attention: q-block rnd for every group ----
            # Both heads of a group advance together through kt-pair units:
            # the two K=64 score matmuls of a pair pack into PE row-bands
            # 0-63 / 64-127 and run concurrently; exp(h0) overlaps AV(h1).
            # Diagonal kt tiles compute only valid query columns, written
            # at packed offsets so one exp instruction covers them.
            attTq = []
            for g in range(NG):
                av = [ps.tile([65, 512], F32, tag=f"av{hh}", name=f"av{hh}")
                      for hh in range(2)]

                # full (below-diagonal) kt tiles: one kt per step, BOTH
                # heads' scores in one [128,1024] psum tile. The two K=64
                # score matmuls are simultaneously ready and adjacent, so
                # they pack into PE row-bands and run concurrently; one
                # exp covers both heads. AV(kt-1) is emitted after
                # scores/exp(kt) (software pipeline skew) so the PE queue
                # never head-blocks on an exp in flight.
                pend = None
                for kt in range(4 * rnd):
                    sc = ps.tile([128, 1024], F32, tag="sc", bufs=2,
                                 name="sc")
                    for hh in range(2):
                        r0 = 64 * hh
                        nc.tensor.matmul(
                            sc[:, hh * 512:(hh + 1) * 512],
                            kT[g][r0:r0 + 64, kt * 128:(kt + 1) * 128],
                            qTq[g][r0:r0 + 64, :],
                            start=True, stop=True,
                            tile_position=(r0, 0),
                        )
                    wT = work.tile([128, 1024], BF, tag="wT", bufs=3,
                                   name="wT")
                    nc.scalar.activation(wT, sc, EXP, scale=SCALE)
                    if pend is not None:
                        pkt, pwT = pend
                        for hh in range(2):
                            nc.tensor.matmul(
                                av[hh][:, 0:512],
                                V[:, pkt, 2 * g + hh, :],
                                pwT[:, hh * 512:(hh + 1) * 512],
                                start=(pkt == 0), stop=False,
                            )
                    pend = (kt, wT)
                if pend is not None:
                    pkt, pwT = pend
                    for hh in range(2):
                        nc.tensor.matmul(
                            av[hh][:, 0:512],
                            V[:, pkt, 2 * g + hh, :],
                            pwT[:, hh * 512:(hh + 1) * 512],
                            start=(pkt == 0), stop=False,
                        )

                # the four diagonal kt tiles, as two kt-pair units with
                # causally-restricted packed columns (per-head sc tiles
                # drawn from the same 2-buffer sc tag)
                for b0 in (4 * rnd, 4 * rnd + 2):
                    jb = b0 - 4 * rnd
                    # (m, q-start, wT col offset, ncols, triangle col or
                    #  None, start, stop) per kt of the unit
                    if jb == 0:
                        plan = [(0, 0, 0, 512, 0, True, True),
                                (1, 128, 512, 384, 512, True, True)]
                        expw = 896
                    else:  # jb == 2: both kts land in psum bank 0 -> one
                        # accumulation group writing disjoint column ranges
                        plan = [(0, 256, 0, 256, 0, True, False),
                                (1, 384, 256, 128, 256, False, True)]
                        expw = 384
                    sc2 = [ps.tile([128, 1024], F32, tag="sc", bufs=2,
                                   name=f"sc{hh}") for hh in range(2)]
                    for (m, qs, co, ncol, tri, st, sp) in plan:
                        for hh in range(2):
                            r0 = 64 * hh
                            nc.tensor.matmul(
                                sc2[hh][:, co:co + ncol],
                                kT[g][r0:r0 + 64,
                                      (b0 + m) * 128:(b0 + m + 1) * 128],
                                qTq[g][r0:r0 + 64, qs:qs + ncol],
                                start=st, stop=sp,
                                tile_position=(r0, 0),
                            )
                    wT2 = [work.tile([128, 1024], BF, tag="wT", bufs=3,
                                     name=f"wTd{hh}") for hh in range(2)]
                    for hh in range(2):
                        nc.scalar.activation(wT2[hh][:, 0:expw],
                                             sc2[hh][:, 0:expw],
                                             EXP, scale=SCALE)
                    for (m, qs, co, ncol, tri, st, sp) in plan:
                        if tri is None:
                            continue
                        for hh in range(2):
                            nc.gpsimd.affine_select(
                                out=wT2[hh][:, tri:tri + 128],
                                in_=wT2[hh][:, tri:tri + 128],
                                compare_op=mybir.AluOpType.is_ge,
                                fill=0.0,
                                base=0,
                                pattern=[[1, 128]],
                                channel_multiplier=-1,
                            )
                    for hh in range(2):
                        head = 2 * g + hh
                        for (m, qs, co, ncol, tri, st, sp) in plan:
                            kt = b0 + m
                            nc.tensor.matmul(
                                av[hh][:, qs:qs + ncol],
                                V[:, kt, head, :],
                                wT2[hh][:, co:co + ncol],
                                start=(kt == 0), stop=(kt == nkt - 1),
                            )
                # stage AV off PSUM and normalize this group right away
                # (keeps the av/avc rotations short and spreads the
                # normalization work across the round)
                att = work.tile([128, 512], BF, tag=f"attTq{g}", bufs=2,
                                name=f"attTq{g}")
                for hh in range(2):
                    avc = work.tile([65, 512], F32, tag="avc", bufs=4,
                                    name="avc")
                    nc.vector.tensor_copy(avc, av[hh])
                    # custom-DVE ops and partition_broadcast only work from
                    # partition base 0, so hop the denominator row from
                    # partition 64 to 0 (HWDGE SBUF->SBUF on the sync ring)
                    den0 = work.tile([1, 512], F32, tag="den0", bufs=4,
                                     name="den0")
                    nc.sync.dma_start(out=den0, in_=avc[64:65, :])
                    den_b = work.tile([64, 512], F32, tag="den_b",
                                      bufs=4, name="den_b")
                    nc.gpsimd.partition_broadcast(out_ap=den_b, in_ap=den0)
                    rep = work.tile([64, 512], F32, tag="rep", bufs=4,
                                    name="rep")
                    nc.vector.reciprocal_approx_fast(out=rep, in_=den_b)
                    if hh == 0:
                        nc.vector.tensor_mul(att[0:64, :], avc[0:64, :], rep)
                    else:
                        tmpB = work.tile([64, 512], BF, tag="tmpB", bufs=2,
                                         name="tmpB")
                        nc.vector.tensor_mul(tmpB, avc[0:64, :], rep)
                        nc.gpsimd.dma_start(out=att[64:128, :], in_=tmpB)
                attTq.append(att)

            prev_attTq = attTq
        emit_out_proj(3, prev_attTq)

    nc.compile()
    return nc


_NC_CACHE = None


def _get_nc():
    global _NC_CACHE
    if _NC_CACHE is None:
        _NC_CACHE = build_nc()
    return _NC_CACHE


def kernel(x, w_qkv, w_out, _trace=False):
    import ml_dtypes

    bf16 = ml_dtypes.bfloat16
    B = x.shape[0]
    x = np.asarray(x, dtype=np.float32)
    w_qkv = np.asarray(w_qkv, dtype=np.float32)
    w_out = np.asarray(w_out, dtype=np.float32)

    nc = _get_nc()
    in_maps = []
    for core in range(8):
        b = core % B
        hbase = (core // B) * HC
        lo, hi = hbase * D, hbase * D + HC * D

        def warr(w):  # [C, 512] -> [128, NCT*512]
            return w.reshape(NCT, 128, 512).transpose(1, 0, 2).reshape(
                128, NCT * 512)

        wo = w_out[lo:hi, :].reshape(NG, 128, C).transpose(1, 0, 2).reshape(
            128, NG * C)  # [512, C] -> [128, NG*C]
        wqk = np.concatenate(
            [warr(w_qkv[:, lo:hi]), warr(w_qkv[:, C + lo:C + hi])], axis=1)
        wvo = np.concatenate(
            [warr(w_qkv[:, 2 * C + lo:2 * C + hi]), wo], axis=1)
        in_maps.append({
            "x": np.ascontiguousarray(x[b].astype(bf16)),
            "wqk": np.ascontiguousarray(wqk.astype(bf16)),
            "wvo": np.ascontiguousarray(wvo.astype(bf16)),
        })

    res = run_bass_kernel_spmd(nc, in_maps, core_ids=list(range(8)), trace=_trace)
    ys = [r["y"] for r in res.results]
    out = np.empty((B, T, C), dtype=np.float32)
    for b in range(B):
        out[b] = ys[b] + ys[b + B]
    if _trace:
        return out, res
    return out


# revision 29
# speedup vs baseline: 1.5281x; 1.0018x over previous
"""Causal self-attention for trn2, 8 NeuronCores.

Problem: x[4,2048,1024] @ w_qkv[1024,3072] -> causal MHA (16 heads, d=64)
-> @ w_out[1024,1024].

Sharding: core c handles batch b=c%4 and heads hbase=8*(c//4)..hbase+8
(data parallel on B x tensor parallel on heads). Each core computes the
partial out-projection y_c = att_slice @ w_out[slice]; the host sums the
two partials per batch.

v6: inputs arrive pre-cast to bf16 and pre-arranged on the host (numpy
round-to-nearest, same numerics as the previous on-chip DVE casts), so
the kernel has no f32 weight loads, no SWDGE cast chain, and every
hardware DMA-transpose of x can start at t=0. DMA traffic is split
across the two HWDGE rings (weights on the scalar ring, x-transposes on
the sync ring) so startup is no longer serialized on one ring; y stores
and SBUF-SBUF moves go through gpsimd SWDGE. Attention processes both
heads of a group per kt-pair unit (K=64 score matmuls at PE row-bands
(0,0)/(64,0), exp(h0) overlaps matmuls(h1)); diagonal kt tiles compute
only causally-valid query columns in a packed layout with a uniform
[128,128] triangle affine_select. Softmax denominators ride a ones-row
in the AV matmul; the den row hops to partition 0 via a tiny SWDGE
move, gpsimd partition_broadcast fans it out (custom-DVE ops and the
broadcast only work from partition base 0), then one
reciprocal_approx_fast + multiply per head. PSUM tags: sc0/sc1 (2
banks each), av0/av1 (1 each), proj (2); out-projection reuses sc.
"""

import sys

for p in ("/opt/trn_rl_repo", "/opt/pypackages"):
    if p not in sys.path:
        sys.path.insert(0, p)

import contextlib

import numpy as np

import concourse.bass as bass
import concourse.mybir as mybir
import concourse.tile as tile
from concourse import bacc
from concourse.bass_utils import run_bass_kernel_spmd

F32 = mybir.dt.float32
BF = mybir.dt.bfloat16
EXP = mybir.ActivationFunctionType.Exp

T = 2048          # sequence length
C = 1024          # model dim
HC = 8            # heads per core
D = 64            # head dim
NG = 4            # head-groups of 2 per core
NCT = C // 128    # 8 contraction tiles
NTT = T // 128    # 16 token tiles
SCALE = 0.125     # 1/sqrt(D)

USE_GPSIMD_BCAST = True


def build_nc():
    nc = bacc.Bacc("TRN2", target_bir_lowering=False, debug=False)

    x_d = nc.dram_tensor("x", [T, C], BF, kind="ExternalInput")
    # weights ship as two host-packed tensors (2 DMAs, fewer DMA-sem
    # lane conflicts at startup): wqk = [wq | wk], wvo = [wv | wo]
    wqk_d = nc.dram_tensor("wqk", [128, 2 * NCT * 512], BF,
                           kind="ExternalInput")
    wvo_d = nc.dram_tensor("wvo", [128, NCT * 512 + NG * C], BF,
                           kind="ExternalInput")
    y_d = nc.dram_tensor("y", [T, C], F32, kind="ExternalOutput")

    with tile.TileContext(nc) as tc, contextlib.ExitStack() as ctx:
        persist = ctx.enter_context(tc.tile_pool(name="persist", bufs=1))
        work = ctx.enter_context(tc.tile_pool(name="work", bufs=1))
        ps = ctx.enter_context(tc.tile_pool(name="ps", bufs=1, space="PSUM"))
        dpool = ctx.enter_context(tc.tile_pool(name="dram", bufs=1, space="DRAM"))

        kT = [persist.tile([128, T], BF, tag=f"kT{g}", name=f"kT{g}")
              for g in range(NG)]
        V = persist.tile([128, NTT, HC, 65], BF, tag="V")

        # weights: two bf16 loads on the scalar HWDGE ring (the sync
        # ring is busy with the x DMA-transposes at startup)
        wqk_sb = persist.tile([128, 2 * NCT * 512], BF, tag="wqk_sb")
        wvo_sb = persist.tile([128, NCT * 512 + NG * C], BF, tag="wvo_sb")
        nc.scalar.dma_start(out=wqk_sb, in_=wqk_d.ap())
        nc.scalar.dma_start(out=wvo_sb, in_=wvo_d.ap())

        def wq_ap(ct, cols):  # [128, 128-slice of the ct-block]
            return wqk_sb[:, ct * 512 + cols[0]:ct * 512 + cols[1]]

        def wk_ap(ct, cols):
            base = NCT * 512
            return wqk_sb[:, base + ct * 512 + cols[0]:base + ct * 512 + cols[1]]

        def wv_ap(ct):
            return wvo_sb[:, ct * 512:(ct + 1) * 512]

        def wo_ap(g, cols):
            base = NCT * 512
            return wvo_sb[:, base + g * C + cols[0]:base + g * C + cols[1]]

        # ones column of V
        ones_f32 = persist.tile([128, NTT, HC], F32, tag="ones")
        nc.vector.memset(ones_f32, 1.0)
        nc.vector.tensor_copy(V[:, :, :, 64], ones_f32)

        def emit_out_proj(rnd_, att_):
            # out projection of round rnd_'s q rows (psum from the proj
            # tag; y stores ride the gpsimd SWDGE so the sync ring stays
            # free for DMA-transposes and denominator hops)
            for qtl in range(4):
                qt = rnd_ * 4 + qtl
                psy = ps.tile([128, 1024], F32, tag="proj", name="psy")
                for g in range(NG):
                    for half in range(2):
                        nc.tensor.matmul(
                            psy[:, half * 512:(half + 1) * 512],
                            att_[g][:, qtl * 128:(qtl + 1) * 128],
                            wo_ap(g, (half * 512, (half + 1) * 512)),
                            start=(g == 0),
                            stop=(g == NG - 1),
                        )
                y_sb = work.tile([128, C], F32, tag="y_sb", bufs=2,
                                 name="y_sb")
                nc.vector.tensor_copy(y_sb, psy)
                nc.sync.dma_start(
                    out=y_d.ap()[qt * 128:(qt + 1) * 128, :], in_=y_sb
                )

        prev_attTq = None
        for rnd in range(4):
            q0 = rnd * 512  # first token of this quarter
            nkt = 4 * (rnd + 1)

            # ---- xT quarter via hardware DMA-transpose ----
            xTq = [work.tile([128, 512], BF, tag=f"xTq{ct}",
                             name=f"xTq{ct}", bufs=2)
                   for ct in range(NCT)]
            for ct in range(NCT):
                nc.sync.dma_start_transpose(
                    out=xTq[ct],
                    in_=x_d.ap()[q0:q0 + 512, ct * 128:(ct + 1) * 128]
                )

            # ---- qT/kT for this quarter ----
            qTq = []
            for g in range(NG):
                pqk = ps.tile([128, 1024], F32, tag="proj", name="pqk")
                # q-chain fully before k-chain so the PE isn't stalled on
                # the wk half of the weight load at startup
                for ct in range(NCT):
                    nc.tensor.matmul(
                        pqk[:, 0:512],
                        wq_ap(ct, (g * 128, (g + 1) * 128)),
                        xTq[ct],
                        start=(ct == 0), stop=(ct == NCT - 1),
                    )
                for ct in range(NCT):
                    nc.tensor.matmul(
                        pqk[:, 512:1024],
                        wk_ap(ct, (g * 128, (g + 1) * 128)),
                        xTq[ct],
                        start=(ct == 0), stop=(ct == NCT - 1),
                    )
                qq = work.tile([128, 512], BF, tag=f"qTq{g}", bufs=2,
                               name=f"qTq{g}")
                nc.vector.tensor_copy(qq, pqk[:, 0:512])
                qTq.append(qq)
                nc.vector.tensor_copy(kT[g][:, q0:q0 + 512], pqk[:, 512:1024])

            # ---- V for this quarter (two tt-pairs per psum tile) ----
            for half in range(2):
                pv = ps.tile([128, 1024], F32, tag="proj", name="pv")
                for ct in range(NCT):
                    for sub in range(2):
                        jl = half * 2 + sub
                        nc.tensor.matmul(
                            pv[:, sub * 512:(sub + 1) * 512],
                            xTq[ct][:, jl * 128:(jl + 1) * 128],
                            wv_ap(ct),
                            start=(ct == 0), stop=(ct == NCT - 1),
                        )
                tt0 = rnd * 4 + half * 2
                nc.vector.tensor_copy(
                    V[:, tt0:tt0 + 2, :, 0:64],
                    pv.rearrange("p (t h d) -> p t h d", t=2, h=HC),
                )

            # ---- attention: q-block rnd for every group ----
            # Both heads of a group advance together through kt-pair units:
            # the two K=64 score matmuls of a pair pack into PE row-bands
            # 0-63 / 64-127 and run concurrently; exp(h0) overlaps AV(h1).
            # Diagonal kt tiles compute only valid query columns, written
            # at packed offsets so one exp instruction covers them.
            attTq = []
            for g in range(NG):
                # the previous round's out-projection is emitted after this
                # round's first attention group: the proj-tag rotation lets
                # it overlap this round's attention, and the PE queue has
                # attention work in front of it while the previous round's
                # last-group normalization drains
                if g == 1 and rnd > 0:
                    emit_out_proj(rnd - 1, prev_attTq)
                av = [ps.tile([65, 512], F32, tag=f"av{hh}", name=f"av{hh}")
                      for hh in range(2)]

                # full (below-diagonal) kt tiles: one kt per step, BOTH
                # heads' scores in one [128,1024] psum tile. The two K=64
                # score matmuls are simultaneously ready and adjacent, so
                # they pack into PE row-bands and run concurrently; one
                # exp covers both heads. AV(kt-1) is emitted after
                # scores/exp(kt) (software pipeline skew) so the PE queue
                # never head-blocks on an exp in flight.
                pend = None
                for kt in range(4 * rnd):
                    sc = ps.tile([128, 1024], F32, tag="sc", bufs=2,
                                 name="sc")
                    for hh in range(2):
                        r0 = 64 * hh
                        nc.tensor.matmul(
                            sc[:, hh * 512:(hh + 1) * 512],
                            kT[g][r0:r0 + 64, kt * 128:(kt + 1) * 128],
                            qTq[g][r0:r0 + 64, :],
                            start=True, stop=True,
                            tile_position=(r0, 0),
                        )
                    wT = work.tile([128, 1024], BF, tag="wT", bufs=3,
                                   name="wT")
                    nc.scalar.activation(wT, sc, EXP, scale=SCALE)
                    if pend is not None:
                        pkt, pwT = pend
                        for hh in range(2):
                            nc.tensor.matmul(
                                av[hh][:, 0:512],
                                V[:, pkt, 2 * g + hh, :],
                                pwT[:, hh * 512:(hh + 1) * 512],
                                start=(pkt == 0), stop=False,
                            )
                    pend = (kt, wT)
                if pend is not None:
                    pkt, pwT = pend
                    for hh in range(2):
                        nc.tensor.matmul(
                            av[hh][:, 0:512],
                            V[:, pkt, 2 * g + hh, :],
                            pwT[:, hh * 512:(hh + 1) * 512],
                            start=(pkt == 0), stop=False,
                        )

                # the four diagonal kt tiles, as two kt-pair units with
                # causally-restricted packed columns (per-head sc tiles
                # drawn from the same 2-buffer sc tag)
                for b0 in (4 * rnd, 4 * rnd + 2):
                    jb = b0 - 4 * rnd
                    # (m, q-start, wT col offset, ncols, triangle col or
                    #  None, start, stop) per kt of the unit
                    if jb == 0:
                        plan = [(0, 0, 0, 512, 0, True, True),
                                (1, 128, 512, 384, 512, True, True)]
                        expw = 896
                    else:  # jb == 2: both kts land in psum bank 0 -> one
                        # accumulation group writing disjoint column ranges
                        plan = [(0, 256, 0, 256, 0, True, False),
                                (1, 384, 256, 128, 256, False, True)]
                        expw = 384
                    sc2 = [ps.tile([128, 1024], F32, tag="sc", bufs=2,
                                   name=f"sc{hh}") for hh in range(2)]
                    for (m, qs, co, ncol, tri, st, sp) in plan:
                        for hh in range(2):
                            r0 = 64 * hh
                            nc.tensor.matmul(
                                sc2[hh][:, co:co + ncol],
                                kT[g][r0:r0 + 64,
                                      (b0 + m) * 128:(b0 + m + 1) * 128],
                                qTq[g][r0:r0 + 64, qs:qs + ncol],
                                start=st, stop=sp,
                                tile_position=(r0, 0),
                            )
                    wT2 = [work.tile([128, 1024], BF, tag="wT", bufs=3,
                                     name=f"wTd{hh}") for hh in range(2)]
                    for hh in range(2):
                        nc.scalar.activation(wT2[hh][:, 0:expw],
                                             sc2[hh][:, 0:expw],
                                             EXP, scale=SCALE)
                    for (m, qs, co, ncol, tri, st, sp) in plan:
                        if tri is None:
                            continue
                        for hh in range(2):
                            nc.gpsimd.affine_select(
                                out=wT2[hh][:, tri:tri + 128],
                                in_=wT2[hh][:, tri:tri + 128],
                                compare_op=mybir.AluOpType.is_ge,
                                fill=0.0,
                                base=0,
                                pattern=[[1, 128]],
                                channel_multiplier=-1,
                            )
                    for hh in range(2):
                        head = 2 * g + hh
                        for (m, qs, co, ncol, tri, st, sp) in plan:
                            kt = b0 + m
                            nc.tensor.matmul(
                                av[hh][:, qs:qs + ncol],
                                V[:, kt, head, :],
                                wT2[hh][:, co:co + ncol],
                                start=(kt == 0), stop=(kt == nkt - 1),
                            )
                # stage AV off PSUM and normalize this group right away
                # (keeps the av/avc rotations short and spreads the
                # normalization work across the round)
                att = work.tile([128, 512], BF, tag=f"attTq{g}", bufs=2,
                                name=f"attTq{g}")
                for hh in range(2):
                    avc = work.tile([65, 512], F32, tag="avc", bufs=4,
                                    name="avc")
                    nc.vector.tensor_copy(avc, av[hh])
                    # custom-DVE ops and partition_broadcast only work from
                    # partition base 0, so hop the denominator row from
                    # partition 64 to 0 (HWDGE SBUF->SBUF on the sync ring)
                    den0 = work.tile([1, 512], F32, tag="den0", bufs=4,
                                     name="den0")
                    nc.sync.dma_start(out=den0, in_=avc[64:65, :])
                    den_b = work.tile([64, 512], F32, tag="den_b",
                                      bufs=4, name="den_b")
                    nc.gpsimd.partition_broadcast(out_ap=den_b, in_ap=den0)
                    rep = work.tile([64, 512], F32, tag="rep", bufs=4,
                                    name="rep")
                    nc.vector.reciprocal_approx_fast(out=rep, in_=den_b)
                    if hh == 0:
                        nc.vector.tensor_mul(att[0:64, :], avc[0:64, :], rep)
                    else:
                        tmpB = work.tile([64, 512], BF, tag="tmpB", bufs=2,
                                         name="tmpB")
                        nc.vector.tensor_mul(tmpB, avc[0:64, :], rep)
                        nc.sync.dma_start(out=att[64:128, :], in_=tmpB)
                attTq.append(att)

            prev_attTq = attTq
        emit_out_proj(3, prev_attTq)

    nc.compile()
    return nc


_NC_CACHE = None


def _get_nc():
    global _NC_CACHE
    if _NC_CACHE is None:
        _NC_CACHE = build_nc()
    return _NC_CACHE


def kernel(x, w_qkv, w_out, _trace=False):
    import ml_dtypes

    bf16 = ml_dtypes.bfloat16
    B = x.shape[0]
    x = np.asarray(x, dtype=np.float32)
    w_qkv = np.asarray(w_qkv, dtype=np.float32)
    w_out = np.asarray(w_out, dtype=np.float32)

    nc = _get_nc()
    in_maps = []
    for core in range(8):
        b = core % B
        hbase = (core // B) * HC
        lo, hi = hbase * D, hbase * D + HC * D

        def warr(w):  # [C, 512] -> [128, NCT*512]
            return w.reshape(NCT, 128, 512).transpose(1, 0, 2).reshape(
                128, NCT * 512)

        wo = w_out[lo:hi, :].reshape(NG, 128, C).transpose(1, 0, 2).reshape(
            128, NG * C)  # [512, C] -> [128, NG*C]
        wqk = np.concatenate(
            [warr(w_qkv[:, lo:hi]), warr(w_qkv[:, C + lo:C + hi])], axis=1)
        wvo = np.concatenate(
            [warr(w_qkv[:, 2 * C + lo:2 * C + hi]), wo], axis=1)
        in_maps.append({
            "x": np.ascontiguousarray(x[b].astype(bf16)),
            "wqk": np.ascontiguousarray(wqk.astype(bf16)),
            "wvo": np.ascontiguousarray(wvo.astype(bf16)),
        })

    res = run_bass_kernel_spmd(nc, in_maps, core_ids=list(range(8)), trace=_trace)
    ys = [r["y"] for r in res.results]
    out = np.empty((B, T, C), dtype=np.float32)
    for b in range(B):
        out[b] = ys[b] + ys[b + B]
    if _trace:
        return out, res
    return out


# revision 30
# speedup vs baseline: 1.5321x; 1.0026x over previous
"""Causal self-attention for trn2, 8 NeuronCores.

Problem: x[4,2048,1024] @ w_qkv[1024,3072] -> causal MHA (16 heads, d=64)
-> @ w_out[1024,1024].

Sharding: core c handles batch b=c%4 and heads hbase=8*(c//4)..hbase+8
(data parallel on B x tensor parallel on heads). Each core computes the
partial out-projection y_c = att_slice @ w_out[slice]; the host sums the
two partials per batch.

v6: inputs arrive pre-cast to bf16 and pre-arranged on the host (numpy
round-to-nearest, same numerics as the previous on-chip DVE casts), so
the kernel has no f32 weight loads, no SWDGE cast chain, and every
hardware DMA-transpose of x can start at t=0. DMA traffic is split
across the two HWDGE rings (weights on the scalar ring, x-transposes on
the sync ring) so startup is no longer serialized on one ring; y stores
and SBUF-SBUF moves go through gpsimd SWDGE. Attention processes both
heads of a group per kt-pair unit (K=64 score matmuls at PE row-bands
(0,0)/(64,0), exp(h0) overlaps matmuls(h1)); diagonal kt tiles compute
only causally-valid query columns in a packed layout with a uniform
[128,128] triangle affine_select. Softmax denominators ride a ones-row
in the AV matmul; the den row hops to partition 0 via a tiny SWDGE
move, gpsimd partition_broadcast fans it out (custom-DVE ops and the
broadcast only work from partition base 0), then one
reciprocal_approx_fast + multiply per head. PSUM tags: sc0/sc1 (2
banks each), av0/av1 (1 each), proj (2); out-projection reuses sc.
"""

import sys

for p in ("/opt/trn_rl_repo", "/opt/pypackages"):
    if p not in sys.path:
        sys.path.insert(0, p)

import contextlib

import numpy as np

import concourse.bass as bass
import concourse.mybir as mybir
import concourse.tile as tile
from concourse import bacc
from concourse.bass_utils import run_bass_kernel_spmd

F32 = mybir.dt.float32
BF = mybir.dt.bfloat16
EXP = mybir.ActivationFunctionType.Exp

T = 2048          # sequence length
C = 1024          # model dim
HC = 8            # heads per core
D = 64            # head dim
NG = 4            # head-groups of 2 per core
NCT = C // 128    # 8 contraction tiles
NTT = T // 128    # 16 token tiles
SCALE = 0.125     # 1/sqrt(D)

USE_GPSIMD_BCAST = True


def build_nc():
    nc = bacc.Bacc("TRN2", target_bir_lowering=False, debug=False)

    x_d = nc.dram_tensor("x", [T, C], BF, kind="ExternalInput")
    # weights ship as two host-packed tensors (2 DMAs, fewer DMA-sem
    # lane conflicts at startup): wqk = [wq | wk], wvo = [wv | wo]
    wqk_d = nc.dram_tensor("wqk", [128, 2 * NCT * 512], BF,
                           kind="ExternalInput")
    wvo_d = nc.dram_tensor("wvo", [128, NCT * 512 + NG * C], BF,
                           kind="ExternalInput")
    y_d = nc.dram_tensor("y", [T, C], F32, kind="ExternalOutput")

    with tile.TileContext(nc) as tc, contextlib.ExitStack() as ctx:
        persist = ctx.enter_context(tc.tile_pool(name="persist", bufs=1))
        work = ctx.enter_context(tc.tile_pool(name="work", bufs=1))
        ps = ctx.enter_context(tc.tile_pool(name="ps", bufs=1, space="PSUM"))
        dpool = ctx.enter_context(tc.tile_pool(name="dram", bufs=1, space="DRAM"))

        kT = [persist.tile([128, T], BF, tag=f"kT{g}", name=f"kT{g}")
              for g in range(NG)]
        V = persist.tile([128, NTT, HC, 65], BF, tag="V")

        # weights: two bf16 loads on the scalar HWDGE ring (the sync
        # ring is busy with the x DMA-transposes at startup)
        wqk_sb = persist.tile([128, 2 * NCT * 512], BF, tag="wqk_sb")
        wvo_sb = persist.tile([128, NCT * 512 + NG * C], BF, tag="wvo_sb")
        nc.scalar.dma_start(out=wqk_sb, in_=wqk_d.ap())
        nc.scalar.dma_start(out=wvo_sb, in_=wvo_d.ap())

        def wq_ap(ct, cols):  # [128, 128-slice of the ct-block]
            return wqk_sb[:, ct * 512 + cols[0]:ct * 512 + cols[1]]

        def wk_ap(ct, cols):
            base = NCT * 512
            return wqk_sb[:, base + ct * 512 + cols[0]:base + ct * 512 + cols[1]]

        def wv_ap(ct):
            return wvo_sb[:, ct * 512:(ct + 1) * 512]

        def wo_ap(g, cols):
            base = NCT * 512
            return wvo_sb[:, base + g * C + cols[0]:base + g * C + cols[1]]

        # ones column of V
        ones_f32 = persist.tile([128, NTT, HC], F32, tag="ones")
        nc.vector.memset(ones_f32, 1.0)
        nc.vector.tensor_copy(V[:, :, :, 64], ones_f32)

        def emit_out_proj(rnd_, att_):
            # out projection of round rnd_'s q rows (psum from the proj
            # tag; y stores ride the gpsimd SWDGE so the sync ring stays
            # free for DMA-transposes and denominator hops)
            for qtl in range(4):
                qt = rnd_ * 4 + qtl
                psy = ps.tile([128, 1024], F32, tag="proj", name="psy")
                for g in range(NG):
                    for half in range(2):
                        nc.tensor.matmul(
                            psy[:, half * 512:(half + 1) * 512],
                            att_[g][:, qtl * 128:(qtl + 1) * 128],
                            wo_ap(g, (half * 512, (half + 1) * 512)),
                            start=(g == 0),
                            stop=(g == NG - 1),
                        )
                y_sb = work.tile([128, C], F32, tag="y_sb", bufs=2,
                                 name="y_sb")
                nc.vector.tensor_copy(y_sb, psy)
                nc.sync.dma_start(
                    out=y_d.ap()[qt * 128:(qt + 1) * 128, :], in_=y_sb
                )

        def emit_transposes(rnd_):
            q0_ = rnd_ * 512
            xq = [work.tile([128, 512], BF, tag=f"xTq{ct}",
                            name=f"xTq{ct}", bufs=2) for ct in range(NCT)]
            for ct in range(NCT):
                nc.sync.dma_start_transpose(
                    out=xq[ct],
                    in_=x_d.ap()[q0_:q0_ + 512, ct * 128:(ct + 1) * 128])
            return xq

        def emit_qk(rnd_, g, xq):
            q0_ = rnd_ * 512
            pqk = ps.tile([128, 1024], F32, tag="proj", name="pqk")
            # q-chain fully before k-chain so the PE isn't stalled on
            # the wk half of the weight load at startup
            for ct in range(NCT):
                nc.tensor.matmul(
                    pqk[:, 0:512],
                    wq_ap(ct, (g * 128, (g + 1) * 128)),
                    xq[ct],
                    start=(ct == 0), stop=(ct == NCT - 1),
                )
            for ct in range(NCT):
                nc.tensor.matmul(
                    pqk[:, 512:1024],
                    wk_ap(ct, (g * 128, (g + 1) * 128)),
                    xq[ct],
                    start=(ct == 0), stop=(ct == NCT - 1),
                )
            qq = work.tile([128, 512], BF, tag=f"qTq{g}", bufs=2,
                           name=f"qTq{g}")
            nc.vector.tensor_copy(qq, pqk[:, 0:512])
            nc.vector.tensor_copy(kT[g][:, q0_:q0_ + 512], pqk[:, 512:1024])
            return qq

        def emit_v(rnd_, half, xq):
            pv = ps.tile([128, 1024], F32, tag="proj", name="pv")
            for ct in range(NCT):
                for sub in range(2):
                    jl = half * 2 + sub
                    nc.tensor.matmul(
                        pv[:, sub * 512:(sub + 1) * 512],
                        xq[ct][:, jl * 128:(jl + 1) * 128],
                        wv_ap(ct),
                        start=(ct == 0), stop=(ct == NCT - 1),
                    )
            tt0 = rnd_ * 4 + half * 2
            nc.vector.tensor_copy(
                V[:, tt0:tt0 + 2, :, 0:64],
                pv.rearrange("p (t h d) -> p t h d", t=2, h=HC),
            )

        def attention_group(rnd_, g, qTq_):
            # Both heads of group g advance together; full kt tiles pack
            # both heads' K=64 score matmuls into one [128,1024] psum tile
            # (concurrent PE row-bands, one exp for both heads); diagonal
            # kt tiles compute only causally-valid query columns in a
            # packed layout with a uniform [128,128] triangle select.
            nkt_ = 4 * (rnd_ + 1)
            av = [ps.tile([65, 512], F32, tag=f"av{hh}", name=f"av{hh}")
                  for hh in range(2)]
            pend = None
            for kt in range(4 * rnd_):
                sc = ps.tile([128, 1024], F32, tag="sc", bufs=2, name="sc")
                for hh in range(2):
                    r0 = 64 * hh
                    nc.tensor.matmul(
                        sc[:, hh * 512:(hh + 1) * 512],
                        kT[g][r0:r0 + 64, kt * 128:(kt + 1) * 128],
                        qTq_[g][r0:r0 + 64, :],
                        start=True, stop=True,
                        tile_position=(r0, 0),
                    )
                wT = work.tile([128, 1024], BF, tag="wT", bufs=3, name="wT")
                nc.scalar.activation(wT, sc, EXP, scale=SCALE)
                if pend is not None:
                    pkt, pwT = pend
                    for hh in range(2):
                        nc.tensor.matmul(
                            av[hh][:, 0:512],
                            V[:, pkt, 2 * g + hh, :],
                            pwT[:, hh * 512:(hh + 1) * 512],
                            start=(pkt == 0), stop=False,
                        )
                pend = (kt, wT)
            if pend is not None:
                pkt, pwT = pend
                for hh in range(2):
                    nc.tensor.matmul(
                        av[hh][:, 0:512],
                        V[:, pkt, 2 * g + hh, :],
                        pwT[:, hh * 512:(hh + 1) * 512],
                        start=(pkt == 0), stop=False,
                    )
            for b0 in (4 * rnd_, 4 * rnd_ + 2):
                jb = b0 - 4 * rnd_
                # (m, q-start, wT col offset, ncols, triangle col or None,
                #  start, stop) per kt of the unit
                if jb == 0:
                    plan = [(0, 0, 0, 512, 0, True, True),
                            (1, 128, 512, 384, 512, True, True)]
                    expw = 896
                else:  # jb == 2: both kts land in psum bank 0 -> one
                    # accumulation group writing disjoint column ranges
                    plan = [(0, 256, 0, 256, 0, True, False),
                            (1, 384, 256, 128, 256, False, True)]
                    expw = 384
                sc2 = [ps.tile([128, 1024], F32, tag="sc", bufs=2,
                               name=f"sc{hh}") for hh in range(2)]
                for (m, qs, co, ncol, tri, st, sp) in plan:
                    for hh in range(2):
                        r0 = 64 * hh
                        nc.tensor.matmul(
                            sc2[hh][:, co:co + ncol],
                            kT[g][r0:r0 + 64,
                                  (b0 + m) * 128:(b0 + m + 1) * 128],
                            qTq_[g][r0:r0 + 64, qs:qs + ncol],
                            start=st, stop=sp,
                            tile_position=(r0, 0),
                        )
                wT2 = [work.tile([128, 1024], BF, tag="wT", bufs=3,
                                 name=f"wTd{hh}") for hh in range(2)]
                for hh in range(2):
                    nc.scalar.activation(wT2[hh][:, 0:expw],
                                         sc2[hh][:, 0:expw],
                                         EXP, scale=SCALE)
                for (m, qs, co, ncol, tri, st, sp) in plan:
                    if tri is None:
                        continue
                    for hh in range(2):
                        nc.gpsimd.affine_select(
                            out=wT2[hh][:, tri:tri + 128],
                            in_=wT2[hh][:, tri:tri + 128],
                            compare_op=mybir.AluOpType.is_ge,
                            fill=0.0,
                            base=0,
                            pattern=[[1, 128]],
                            channel_multiplier=-1,
                        )
                for hh in range(2):
                    head = 2 * g + hh
                    for (m, qs, co, ncol, tri, st, sp) in plan:
                        kt = b0 + m
                        nc.tensor.matmul(
                            av[hh][:, qs:qs + ncol],
                            V[:, kt, head, :],
                            wT2[hh][:, co:co + ncol],
                            start=(kt == 0), stop=(kt == nkt_ - 1),
                        )
            # stage AV off PSUM and normalize this group right away
            att = work.tile([128, 512], BF, tag=f"attTq{g}", bufs=2,
                            name=f"attTq{g}")
            for hh in range(2):
                avc = work.tile([65, 512], F32, tag="avc", bufs=4,
                                name="avc")
                nc.vector.tensor_copy(avc, av[hh])
                # custom-DVE ops and partition_broadcast only work from
                # partition base 0, so hop the denominator row from
                # partition 64 to 0 (HWDGE SBUF->SBUF on the sync ring)
                den0 = work.tile([1, 512], F32, tag="den0", bufs=4,
                                 name="den0")
                nc.sync.dma_start(out=den0, in_=avc[64:65, :])
                den_b = work.tile([64, 512], F32, tag="den_b", bufs=4,
                                  name="den_b")
                nc.gpsimd.partition_broadcast(out_ap=den_b, in_ap=den0)
                rep = work.tile([64, 512], F32, tag="rep", bufs=4,
                                name="rep")
                nc.vector.reciprocal_approx_fast(out=rep, in_=den_b)
                if hh == 0:
                    nc.vector.tensor_mul(att[0:64, :], avc[0:64, :], rep)
                else:
                    tmpB = work.tile([64, 512], BF, tag="tmpB", bufs=2,
                                     name="tmpB")
                    nc.vector.tensor_mul(tmpB, avc[0:64, :], rep)
                    nc.sync.dma_start(out=att[64:128, :], in_=tmpB)
            return att

        # ---- software-pipelined emission across rounds ----
        # Round r+1's projections and round r-1's out-projection are
        # emitted INSIDE round r's attention group loop, so every engine
        # queue (PE, DVE, rings) interleaves next-round prep with this
        # round's attention instead of bunching it at round boundaries.
        xTq_r = {0: emit_transposes(0)}
        xTq_r[1] = emit_transposes(1)
        qTq_r = {0: [emit_qk(0, g, xTq_r[0]) for g in range(NG)]}
        for half in range(2):
            emit_v(0, half, xTq_r[0])
        attTq_prev = None
        for rnd in range(4):
            attTq = []
            for g in range(NG):
                attTq.append(attention_group(rnd, g, qTq_r[rnd]))
                if g == 0 and rnd > 0:
                    emit_out_proj(rnd - 1, attTq_prev)
                if rnd < 3:
                    nxt = rnd + 1
                    if g == 0:
                        qTq_r[nxt] = [emit_qk(nxt, 0, xTq_r[nxt]),
                                      emit_qk(nxt, 1, xTq_r[nxt])]
                    elif g == 1:
                        qTq_r[nxt].append(emit_qk(nxt, 2, xTq_r[nxt]))
                        qTq_r[nxt].append(emit_qk(nxt, 3, xTq_r[nxt]))
                    elif g == 2:
                        emit_v(nxt, 0, xTq_r[nxt])
                        emit_v(nxt, 1, xTq_r[nxt])
                    elif g == 3 and rnd < 2:
                        xTq_r[rnd + 2] = emit_transposes(rnd + 2)
            attTq_prev = attTq
        emit_out_proj(3, attTq_prev)

    nc.compile()
    return nc


_NC_CACHE = None


def _get_nc():
    global _NC_CACHE
    if _NC_CACHE is None:
        _NC_CACHE = build_nc()
    return _NC_CACHE


def kernel(x, w_qkv, w_out, _trace=False):
    import ml_dtypes

    bf16 = ml_dtypes.bfloat16
    B = x.shape[0]
    x = np.asarray(x, dtype=np.float32)
    w_qkv = np.asarray(w_qkv, dtype=np.float32)
    w_out = np.asarray(w_out, dtype=np.float32)

    nc = _get_nc()
    in_maps = []
    for core in range(8):
        b = core % B
        hbase = (core // B) * HC
        lo, hi = hbase * D, hbase * D + HC * D

        def warr(w):  # [C, 512] -> [128, NCT*512]
            return w.reshape(NCT, 128, 512).transpose(1, 0, 2).reshape(
                128, NCT * 512)

        wo = w_out[lo:hi, :].reshape(NG, 128, C).transpose(1, 0, 2).reshape(
            128, NG * C)  # [512, C] -> [128, NG*C]
        wqk = np.concatenate(
            [warr(w_qkv[:, lo:hi]), warr(w_qkv[:, C + lo:C + hi])], axis=1)
        wvo = np.concatenate(
            [warr(w_qkv[:, 2 * C + lo:2 * C + hi]), wo], axis=1)
        in_maps.append({
            "x": np.ascontiguousarray(x[b].astype(bf16)),
            "wqk": np.ascontiguousarray(wqk.astype(bf16)),
            "wvo": np.ascontiguousarray(wvo.astype(bf16)),
        })

    res = run_bass_kernel_spmd(nc, in_maps, core_ids=list(range(8)), trace=_trace)
    ys = [r["y"] for r in res.results]
    out = np.empty((B, T, C), dtype=np.float32)
    for b in range(B):
        out[b] = ys[b] + ys[b + B]
    if _trace:
        return out, res
    return out


# revision 34
# speedup vs baseline: 1.5428x; 1.0070x over previous
"""Causal self-attention for trn2, 8 NeuronCores.

Problem: x[4,2048,1024] @ w_qkv[1024,3072] -> causal MHA (16 heads, d=64)
-> @ w_out[1024,1024].

Sharding: core c handles batch b=c%4 and heads hbase=8*(c//4)..hbase+8
(data parallel on B x tensor parallel on heads). Each core computes the
partial out-projection y_c = att_slice @ w_out[slice]; the host sums the
two partials per batch.

v11: inputs arrive pre-cast to bf16 and host-packed (x natural, weights
as two concatenated tensors) so there is no on-chip cast chain and the
x DMA-transposes start at t=0. DMA traffic is split across the two
HWDGE rings (weights on the scalar ring; transposes, denominator hops,
y stores on the sync ring). Emission is software-pipelined across
rounds: round r+1's qk/V projections and round r-1's out-projection are
emitted inside round r's attention group loop so every engine queue
interleaves next-round prep with attention. Attention packs BOTH heads'
K=64 score matmuls for one kt tile into a single [128,1024] psum tile -
the two matmuls land in PE row-bands (0,0)/(64,0), become ready
together, and run concurrently; one exp covers both heads. Diagonal kt
tiles compute only causally-valid query columns in a packed layout with
a uniform [128,128] triangle affine_select, and their AV matmuls write
only the valid column range. Softmax denominators ride a ones-row in
the AV matmul; the den row hops to partition 0 (custom-DVE ops and
gpsimd partition_broadcast only work from partition base 0), is
broadcast by gpsimd, then one reciprocal_approx_fast + multiply per
head. PSUM tags: sc (2 banks x 2 bufs), av0/av1 (1 each), proj (2,
shared by qk/V projections and the deferred out-projection).
USE_FP8_AV (off): an experimental fp8 DoubleRow path for the
below-diagonal AV pairs; produced NaNs on hardware, kept for reference.
"""

import sys

for p in ("/opt/trn_rl_repo", "/opt/pypackages"):
    if p not in sys.path:
        sys.path.insert(0, p)

import contextlib

import numpy as np

import concourse.bass as bass
import concourse.mybir as mybir
import concourse.tile as tile
from concourse import bacc
from concourse.bass_utils import run_bass_kernel_spmd

F32 = mybir.dt.float32
BF = mybir.dt.bfloat16
FP8 = mybir.dt.float8e4
EXP = mybir.ActivationFunctionType.Exp

T = 2048          # sequence length
C = 1024          # model dim
HC = 8            # heads per core
D = 64            # head dim
NG = 4            # head-groups of 2 per core
NCT = C // 128    # 8 contraction tiles
NTT = T // 128    # 16 token tiles
SCALE = 0.125     # 1/sqrt(D)

USE_GPSIMD_BCAST = True
USE_FP8_AV = False


def build_nc():
    nc = bacc.Bacc("TRN2", target_bir_lowering=False, debug=False)

    x_d = nc.dram_tensor("x", [T, C], BF, kind="ExternalInput")
    # weights ship as two host-packed tensors (2 DMAs, fewer DMA-sem
    # lane conflicts at startup): wqk = [wq | wk], wvo = [wv | wo]
    wqk_d = nc.dram_tensor("wqk", [128, 2 * NCT * 512], BF,
                           kind="ExternalInput")
    wvo_d = nc.dram_tensor("wvo", [128, NCT * 512 + NG * C], BF,
                           kind="ExternalInput")
    y_d = nc.dram_tensor("y", [T, C], F32, kind="ExternalOutput")

    with tile.TileContext(nc) as tc, contextlib.ExitStack() as ctx:
        persist = ctx.enter_context(tc.tile_pool(name="persist", bufs=1))
        work = ctx.enter_context(tc.tile_pool(name="work", bufs=1))
        ps = ctx.enter_context(tc.tile_pool(name="ps", bufs=1, space="PSUM"))
        dpool = ctx.enter_context(tc.tile_pool(name="dram", bufs=1, space="DRAM"))

        kT = [persist.tile([128, T], BF, tag=f"kT{g}", name=f"kT{g}")
              for g in range(NG)]
        V = persist.tile([128, NTT, HC, 65], BF, tag="V")
        if USE_FP8_AV:
            # fp8 copy of V interleaved in kt-pairs for DoubleRow AV
            # matmuls (65 -> 80 pads the interleave step to 16B alignment)
            Vp = persist.tile([128, NTT // 2, HC, 2, 80], FP8, tag="Vp")

        # weights: two bf16 loads on the scalar HWDGE ring (the sync
        # ring is busy with the x DMA-transposes at startup)
        wqk_sb = persist.tile([128, 2 * NCT * 512], BF, tag="wqk_sb")
        wvo_sb = persist.tile([128, NCT * 512 + NG * C], BF, tag="wvo_sb")
        nc.scalar.dma_start(out=wqk_sb, in_=wqk_d.ap())
        nc.scalar.dma_start(out=wvo_sb, in_=wvo_d.ap())

        def wq_ap(ct, cols):  # [128, 128-slice of the ct-block]
            return wqk_sb[:, ct * 512 + cols[0]:ct * 512 + cols[1]]

        def wk_ap(ct, cols):
            base = NCT * 512
            return wqk_sb[:, base + ct * 512 + cols[0]:base + ct * 512 + cols[1]]

        def wv_ap(ct):
            return wvo_sb[:, ct * 512:(ct + 1) * 512]

        def wo_ap(g, cols):
            base = NCT * 512
            return wvo_sb[:, base + g * C + cols[0]:base + g * C + cols[1]]

        # ones column of V
        ones_f32 = persist.tile([128, NTT, HC], F32, tag="ones")
        nc.vector.memset(ones_f32, 1.0)
        nc.vector.tensor_copy(V[:, :, :, 64], ones_f32)
        if USE_FP8_AV:
            onesp = persist.tile([128, NTT // 2, HC, 2], F32, tag="onesp")
            nc.vector.memset(onesp, 1.0)
            nc.vector.tensor_copy(Vp[:, :, :, :, 64], onesp)

        def emit_out_proj(rnd_, att_):
            # out projection of round rnd_'s q rows (psum from the proj
            # tag; y stores ride the gpsimd SWDGE so the sync ring stays
            # free for DMA-transposes and denominator hops)
            for qtl in range(4):
                qt = rnd_ * 4 + qtl
                psy = ps.tile([128, 1024], F32, tag="proj", name="psy")
                for g in range(NG):
                    for half in range(2):
                        nc.tensor.matmul(
                            psy[:, half * 512:(half + 1) * 512],
                            att_[g][:, qtl * 128:(qtl + 1) * 128],
                            wo_ap(g, (half * 512, (half + 1) * 512)),
                            start=(g == 0),
                            stop=(g == NG - 1),
                        )
                y_sb = work.tile([128, C], F32, tag="y_sb", bufs=2,
                                 name="y_sb")
                nc.vector.tensor_copy(y_sb, psy)
                nc.sync.dma_start(
                    out=y_d.ap()[qt * 128:(qt + 1) * 128, :], in_=y_sb
                )

        def emit_transposes(rnd_):
            q0_ = rnd_ * 512
            xq = [work.tile([128, 512], BF, tag=f"xTq{ct}",
                            name=f"xTq{ct}", bufs=2) for ct in range(NCT)]
            for ct in range(NCT):
                nc.sync.dma_start_transpose(
                    out=xq[ct],
                    in_=x_d.ap()[q0_:q0_ + 512, ct * 128:(ct + 1) * 128])
            return xq

        def emit_qk(rnd_, g, xq):
            q0_ = rnd_ * 512
            pqk = ps.tile([128, 1024], F32, tag="proj", name="pqk")
            # q-chain fully before k-chain so the PE isn't stalled on
            # the wk half of the weight load at startup
            for ct in range(NCT):
                nc.tensor.matmul(
                    pqk[:, 0:512],
                    wq_ap(ct, (g * 128, (g + 1) * 128)),
                    xq[ct],
                    start=(ct == 0), stop=(ct == NCT - 1),
                )
            for ct in range(NCT):
                nc.tensor.matmul(
                    pqk[:, 512:1024],
                    wk_ap(ct, (g * 128, (g + 1) * 128)),
                    xq[ct],
                    start=(ct == 0), stop=(ct == NCT - 1),
                )
            qq = work.tile([128, 512], BF, tag=f"qTq{g}", bufs=2,
                           name=f"qTq{g}")
            nc.vector.tensor_copy(qq, pqk[:, 0:512])
            nc.vector.tensor_copy(kT[g][:, q0_:q0_ + 512], pqk[:, 512:1024])
            return qq

        def emit_v(rnd_, half, xq):
            pv = ps.tile([128, 1024], F32, tag="proj", name="pv")
            for ct in range(NCT):
                for sub in range(2):
                    jl = half * 2 + sub
                    nc.tensor.matmul(
                        pv[:, sub * 512:(sub + 1) * 512],
                        xq[ct][:, jl * 128:(jl + 1) * 128],
                        wv_ap(ct),
                        start=(ct == 0), stop=(ct == NCT - 1),
                    )
            tt0 = rnd_ * 4 + half * 2
            nc.vector.tensor_copy(
                V[:, tt0:tt0 + 2, :, 0:64],
                pv.rearrange("p (t h d) -> p t h d", t=2, h=HC),
            )
            if USE_FP8_AV:
                # same data, (h, j, d) order, fp8, into the pair tile
                nc.vector.tensor_copy(
                    Vp[:, tt0 // 2, :, :, 0:64],
                    pv.rearrange("p (t h d) -> p h t d", t=2, h=HC),
                )

        def attention_group(rnd_, g, qTq_):
            # Both heads of group g advance together; full kt tiles pack
            # both heads' K=64 score matmuls into one [128,1024] psum tile
            # (concurrent PE row-bands, one exp for both heads); diagonal
            # kt tiles compute only causally-valid query columns in a
            # packed layout with a uniform [128,128] triangle select.
            nkt_ = 4 * (rnd_ + 1)
            av = [ps.tile([65, 512], F32, tag=f"av{hh}", name=f"av{hh}")
                  for hh in range(2)]
            if USE_FP8_AV:
                # full kt tiles in PAIRS: exps write the pair's weights in
                # fp8 into one [128,2048] tile; one DoubleRow matmul per
                # head contracts both kt tiles at once (2 fp8 weights per
                # PE cell)
                pend = None
                for ktp in range(2 * rnd_):
                    wTp = work.tile([128, 2048], FP8, tag="wTp", bufs=3,
                                    name="wTp")
                    for j in range(2):
                        kt = 2 * ktp + j
                        sc = ps.tile([128, 1024], F32, tag="sc", bufs=2,
                                     name="sc")
                        for hh in range(2):
                            r0 = 64 * hh
                            nc.tensor.matmul(
                                sc[:, hh * 512:(hh + 1) * 512],
                                kT[g][r0:r0 + 64, kt * 128:(kt + 1) * 128],
                                qTq_[g][r0:r0 + 64, :],
                                start=True, stop=True,
                                tile_position=(r0, 0),
                            )
                        nc.scalar.activation(
                            wTp[:, j * 1024:(j + 1) * 1024], sc,
                            EXP, scale=SCALE)
                        if j == 0 and pend is not None:
                            pktp, pwTp = pend
                            for hh in range(2):
                                nc.tensor.matmul(
                                    av[hh][:, 0:512],
                                    Vp[:, pktp, 2 * g + hh, :, 0:65],
                                    bass.AP(pwTp.tensor, pwTp.offset
                                            + hh * 512,
                                            [[2048, 128], [1024, 2],
                                             [1, 512]]),
                                    start=(pktp == 0), stop=False,
                                    perf_mode=mybir.MatmulPerfMode.DoubleRow,
                                )
                    pend = (ktp, wTp)
                if pend is not None:
                    pktp, pwTp = pend
                    for hh in range(2):
                        nc.tensor.matmul(
                            av[hh][:, 0:512],
                            Vp[:, pktp, 2 * g + hh, :, 0:65],
                            bass.AP(pwTp.tensor, pwTp.offset + hh * 512,
                                    [[2048, 128], [1024, 2], [1, 512]]),
                            start=(pktp == 0), stop=False,
                            perf_mode=mybir.MatmulPerfMode.DoubleRow,
                        )
                first_diag_kt_starts = (rnd_ == 0)
            else:
                pend = None
                for kt in range(4 * rnd_):
                    sc = ps.tile([128, 1024], F32, tag="sc", bufs=2,
                                 name="sc")
                    for hh in range(2):
                        r0 = 64 * hh
                        nc.tensor.matmul(
                            sc[:, hh * 512:(hh + 1) * 512],
                            kT[g][r0:r0 + 64, kt * 128:(kt + 1) * 128],
                            qTq_[g][r0:r0 + 64, :],
                            start=True, stop=True,
                            tile_position=(r0, 0),
                        )
                    wT = work.tile([128, 1024], BF, tag="wT", bufs=3,
                                   name="wT")
                    nc.scalar.activation(wT, sc, EXP, scale=SCALE)
                    if pend is not None:
                        pkt, pwT = pend
                        for hh in range(2):
                            nc.tensor.matmul(
                                av[hh][:, 0:512],
                                V[:, pkt, 2 * g + hh, :],
                                pwT[:, hh * 512:(hh + 1) * 512],
                                start=(pkt == 0), stop=False,
                            )
                    pend = (kt, wT)
                if pend is not None:
                    pkt, pwT = pend
                    for hh in range(2):
                        nc.tensor.matmul(
                            av[hh][:, 0:512],
                            V[:, pkt, 2 * g + hh, :],
                            pwT[:, hh * 512:(hh + 1) * 512],
                            start=(pkt == 0), stop=False,
                        )
            for b0 in (4 * rnd_, 4 * rnd_ + 2):
                jb = b0 - 4 * rnd_
                # (m, q-start, wT col offset, ncols, triangle col or None,
                #  start, stop) per kt of the unit
                if jb == 0:
                    plan = [(0, 0, 0, 512, 0, True, True),
                            (1, 128, 512, 384, 512, True, True)]
                    expw = 896
                else:  # jb == 2: both kts land in psum bank 0 -> one
                    # accumulation group writing disjoint column ranges
                    plan = [(0, 256, 0, 256, 0, True, False),
                            (1, 384, 256, 128, 256, False, True)]
                    expw = 384
                sc2 = [ps.tile([128, 1024], F32, tag="sc", bufs=2,
                               name=f"sc{hh}") for hh in range(2)]
                for (m, qs, co, ncol, tri, st, sp) in plan:
                    for hh in range(2):
                        r0 = 64 * hh
                        nc.tensor.matmul(
                            sc2[hh][:, co:co + ncol],
                            kT[g][r0:r0 + 64,
                                  (b0 + m) * 128:(b0 + m + 1) * 128],
                            qTq_[g][r0:r0 + 64, qs:qs + ncol],
                            start=st, stop=sp,
                            tile_position=(r0, 0),
                        )
                wT2 = [work.tile([128, 1024], BF, tag="wT", bufs=3,
                                 name=f"wTd{hh}") for hh in range(2)]
                for hh in range(2):
                    nc.scalar.activation(wT2[hh][:, 0:expw],
                                         sc2[hh][:, 0:expw],
                                         EXP, scale=SCALE)
                for (m, qs, co, ncol, tri, st, sp) in plan:
                    if tri is None:
                        continue
                    for hh in range(2):
                        nc.gpsimd.affine_select(
                            out=wT2[hh][:, tri:tri + 128],
                            in_=wT2[hh][:, tri:tri + 128],
                            compare_op=mybir.AluOpType.is_ge,
                            fill=0.0,
                            base=0,
                            pattern=[[1, 128]],
                            channel_multiplier=-1,
                        )
                for hh in range(2):
                    head = 2 * g + hh
                    for (m, qs, co, ncol, tri, st, sp) in plan:
                        kt = b0 + m
                        nc.tensor.matmul(
                            av[hh][:, qs:qs + ncol],
                            V[:, kt, head, :],
                            wT2[hh][:, co:co + ncol],
                            start=(kt == 0), stop=(kt == nkt_ - 1),
                        )
            # stage AV off PSUM and normalize this group right away
            att = work.tile([128, 512], BF, tag=f"attTq{g}", bufs=2,
                            name=f"attTq{g}")
            for hh in range(2):
                avc = work.tile([65, 512], F32, tag="avc", bufs=4,
                                name="avc")
                nc.vector.tensor_copy(avc, av[hh])
                # custom-DVE ops and partition_broadcast only work from
                # partition base 0, so hop the denominator row from
                # partition 64 to 0 (HWDGE SBUF->SBUF on the sync ring)
                den0 = work.tile([1, 512], F32, tag="den0", bufs=4,
                                 name="den0")
                nc.sync.dma_start(out=den0, in_=avc[64:65, :])
                den_b = work.tile([64, 512], F32, tag="den_b", bufs=4,
                                  name="den_b")
                nc.gpsimd.partition_broadcast(out_ap=den_b, in_ap=den0)
                rep = work.tile([64, 512], F32, tag="rep", bufs=4,
                                name="rep")
                nc.vector.reciprocal_approx_fast(out=rep, in_=den_b)
                if hh == 0:
                    nc.vector.tensor_mul(att[0:64, :], avc[0:64, :], rep)
                else:
                    tmpB = work.tile([64, 512], BF, tag="tmpB", bufs=2,
                                     name="tmpB")
                    nc.vector.tensor_mul(tmpB, avc[0:64, :], rep)
                    nc.sync.dma_start(out=att[64:128, :], in_=tmpB)
            return att

        # ---- software-pipelined emission across rounds ----
        # Round r+1's projections and round r-1's out-projection are
        # emitted INSIDE round r's attention group loop, so every engine
        # queue (PE, DVE, rings) interleaves next-round prep with this
        # round's attention instead of bunching it at round boundaries.
        xTq_r = {0: emit_transposes(0)}
        xTq_r[1] = emit_transposes(1)
        qTq_r = {0: [emit_qk(0, g, xTq_r[0]) for g in range(NG)]}
        for half in range(2):
            emit_v(0, half, xTq_r[0])
        attTq_prev = None
        for rnd in range(4):
            attTq = []
            for g in range(NG):
                attTq.append(attention_group(rnd, g, qTq_r[rnd]))
                if g == 0 and rnd > 0:
                    emit_out_proj(rnd - 1, attTq_prev)
                if rnd < 3:
                    nxt = rnd + 1
                    if g == 0:
                        qTq_r[nxt] = [emit_qk(nxt, 0, xTq_r[nxt]),
                                      emit_qk(nxt, 1, xTq_r[nxt])]
                    elif g == 1:
                        qTq_r[nxt].append(emit_qk(nxt, 2, xTq_r[nxt]))
                        qTq_r[nxt].append(emit_qk(nxt, 3, xTq_r[nxt]))
                    elif g == 2:
                        emit_v(nxt, 0, xTq_r[nxt])
                        emit_v(nxt, 1, xTq_r[nxt])
                    elif g == 3 and rnd < 2:
                        xTq_r[rnd + 2] = emit_transposes(rnd + 2)
            attTq_prev = attTq
        emit_out_proj(3, attTq_prev)

    nc.compile()
    return nc


_NC_CACHE = None


def _get_nc():
    global _NC_CACHE
    if _NC_CACHE is None:
        _NC_CACHE = build_nc()
    return _NC_CACHE


def kernel(x, w_qkv, w_out, _trace=False):
    import ml_dtypes

    bf16 = ml_dtypes.bfloat16
    B = x.shape[0]
    x = np.asarray(x, dtype=np.float32)
    w_qkv = np.asarray(w_qkv, dtype=np.float32)
    w_out = np.asarray(w_out, dtype=np.float32)

    nc = _get_nc()
    in_maps = []
    for core in range(8):
        b = core % B
        hbase = (core // B) * HC
        lo, hi = hbase * D, hbase * D + HC * D

        def warr(w):  # [C, 512] -> [128, NCT*512]
            return w.reshape(NCT, 128, 512).transpose(1, 0, 2).reshape(
                128, NCT * 512)

        wo = w_out[lo:hi, :].reshape(NG, 128, C).transpose(1, 0, 2).reshape(
            128, NG * C)  # [512, C] -> [128, NG*C]
        wqk = np.concatenate(
            [warr(w_qkv[:, lo:hi]), warr(w_qkv[:, C + lo:C + hi])], axis=1)
        wvo = np.concatenate(
            [warr(w_qkv[:, 2 * C + lo:2 * C + hi]), wo], axis=1)
        in_maps.append({
            "x": np.ascontiguousarray(x[b].astype(bf16)),
            "wqk": np.ascontiguousarray(wqk.astype(bf16)),
            "wvo": np.ascontiguousarray(wvo.astype(bf16)),
        })

    res = run_bass_kernel_spmd(nc, in_maps, core_ids=list(range(8)), trace=_trace)
    ys = [r["y"] for r in res.results]
    out = np.empty((B, T, C), dtype=np.float32)
    for b in range(B):
        out[b] = ys[b] + ys[b + B]
    if _trace:
        return out, res
    return out
